# revision 1
# baseline (speedup 1.0000x reference)
"""NonLocalAttentionStack kernel for 8 Trainium2 NeuronCores.

Sharding: 8 cores = 4 frames x 2 head-pairs. Each core runs the grouped
Conv3d projection (the dominant-FLOP stage) for its (frame, head-pair)
slice as a chain of PSUM-accumulated matmuls over (kpass x 3x3-offset)
with spatially padded, shifted rhs access patterns. The search / top-k /
gather stages that build the conv input run on host.
"""
import numpy as np

NHEADS, WS, PS, K = 4, 7, 3, 16
B, T, C, H, W = 1, 4, 128, 96, 96
HD_C = C // NHEADS  # 32
PADH, PADW = H + 2, W + 2  # 98, conv3d spatial pad=1
SPAD = PADH * PADW  # 9604
NKP = 8  # contraction passes: 2 heads x (512/128)
NMM = NKP * 9  # 72 accumulating matmuls per output chunk
YCH = 4  # y-rows per output chunk
NCHUNK = H // YCH  # 24
NFREE = YCH * W  # 384


def _host_pre(vid, ln_w, ln_b, wq, bq, wk, bk, wv, bv):
    """LN + QKV + non-local search + topk + softmax + stack (numpy, fp32).

    Returns stack of shape (B*T, C, K, H, W) matching reference.nl_stack.
    """
    vid = np.asarray(vid, np.float32)
    mu = vid.mean(axis=2, keepdims=True)
    var = vid.var(axis=2, keepdims=True)
    x = (vid - mu) / np.sqrt(var + 1e-6)
    x = x * ln_w[None, None, :, None, None] + ln_b[None, None, :, None, None]

    def conv1x1(w, b):
        return np.einsum('btchw,oc->btohw', x, w,
                         optimize=True) + b[None, None, :, None, None]

    q = conv1x1(wq, bq).reshape(B, T, NHEADS, HD_C, H, W)
    k = conv1x1(wk, bk).reshape(B, T, NHEADS, HD_C, H, W)
    v = conv1x1(wv, bv).reshape(B, T, NHEADS, HD_C, H, W)

    r = WS // 2
    kp = np.pad(k, ((0, 0),) * 4 + ((r, r), (r, r)))
    # pixel inner products for all 49 offsets, then 3x3 box sum (zero pad)
    scores = np.empty((B, T, NHEADS, H, W, WS * WS), np.float32)
    for o in range(WS * WS):
        dy, dx = o // WS, o % WS
        s = np.einsum('bthcij,bthcij->bthij', q,
                      kp[:, :, :, :, dy:dy + H, dx:dx + W], optimize=True)
        sp = np.pad(s, ((0, 0),) * 3 + ((1, 1), (1, 1)))
        bs = np.zeros_like(s)
        for ddy in range(PS):
            for ddx in range(PS):
                bs += sp[:, :, :, ddy:ddy + H, ddx:ddx + W]
        scores[..., o] = bs
    # top-16 of 49, sorted desc, ties -> lowest index (match lax.top_k)
    order = np.argsort(-scores, axis=-1, kind='stable')[..., :K]
    dists = np.take_along_axis(scores, order, axis=-1)
    inds = order.astype(np.int32)
    m = dists.max(axis=-1, keepdims=True)
    e = np.exp(dists - m)
    weights = (e / e.sum(axis=-1, keepdims=True)).astype(np.float32)

    vp = np.pad(v, ((0, 0),) * 4 + ((r, r), (r, r)))
    Hp, Wp = H + 2 * r, W + 2 * r
    row = np.arange(H)[:, None, None] + inds // WS       # (B,T,HD,H,W,K)
    col = np.arange(W)[None, :, None] + inds % WS
    lin = (row * Wp + col).reshape(B, T, NHEADS, 1, H * W * K)
    vf = vp.reshape(B, T, NHEADS, HD_C, Hp * Wp)
    g = np.take_along_axis(
        vf, np.broadcast_to(lin, (B, T, NHEADS, HD_C, H * W * K)), axis=-1)
    g = g.reshape(B, T, NHEADS, HD_C, H, W, K)
    g = g * weights[:, :, :, None]
    return np.transpose(g, (0, 1, 2, 3, 6, 4, 5)).reshape(
        B * T, C, K, H, W).astype(np.float32)


def _build_core_inputs(stack, proj_w, bf16):
    """Per-core G (padded, kpass-major) and lhsT weight tables."""
    in_maps = []
    for core in range(8):
        t, hp = core // 2, core % 2
        G = np.zeros((NKP, 128, SPAD), np.float32)
        for side in range(2):          # head A / head B of the pair
            h = hp * 2 + side
            # (HD_C, K, H, W) -> ik = k*32+i major
            s = stack[t, h * HD_C:(h + 1) * HD_C]      # (32, K, H, W)
            s = np.transpose(s, (1, 0, 2, 3)).reshape(512, H, W)
            pad = np.zeros((512, PADH, PADW), np.float32)
            pad[:, 1:1 + H, 1:1 + W] = s
            pad = pad.reshape(4, 128, SPAD)
            G[side * 4:(side + 1) * 4] = pad
        Gf = np.ascontiguousarray(
            np.transpose(G, (1, 0, 2)).reshape(128, NKP * SPAD))

        LT = np.zeros((128, NMM * 64), np.float32)
        for p in range(NKP):
            side, pl = p // 4, p % 4
            for d in range(9):
                dy, dx = d // 3, d % 3
                m = p * 9 + d
                ik = pl * 128 + np.arange(128)
                kk, ii = ik // 32, ik % 32
                ocs = np.arange(32) + side * 32       # cols for this head
                og = hp * 64 + ocs                    # global out channel
                LT[:, m * 64 + ocs] = proj_w[og[None, :], ii[:, None],
                                             kk[:, None], dy, dx]
        in_maps.append({'g': Gf.astype(bf16), 'lt': LT.astype(bf16)})
    return in_maps


def _build_bass():
    import concourse.bacc as bacc
    import concourse.mybir as mybir
    from concourse.tile import TileContext

    nc = bacc.Bacc()
    g = nc.declare_dram_parameter('g', [128, NKP * SPAD], mybir.dt.bfloat16,
                                  isOutput=False)
    lt = nc.declare_dram_parameter('lt', [128, NMM * 64], mybir.dt.bfloat16,
                                   isOutput=False)
    out = nc.declare_dram_parameter('out', [64, H, W], mybir.dt.float32,
                                    isOutput=True)
    with TileContext(nc) as tc:
        with (
            tc.tile_pool(name='gp', bufs=1) as gp,
            tc.tile_pool(name='wp', bufs=1) as wp,
            tc.tile_pool(name='bp', bufs=3) as bp,
            tc.tile_pool(name='pp', bufs=2, space='PSUM') as pp,
        ):
            gsb = gp.tile([128, NKP * SPAD], mybir.dt.bfloat16)
            ltsb = wp.tile([128, NMM * 64], mybir.dt.bfloat16)
            nc.sync.dma_start(out=gsb[:, :], in_=g[:, :])
            nc.sync.dma_start(out=ltsb[:, :], in_=lt[:, :])
            gv = gsb[:, :].rearrange('p (k y x) -> p k y x', k=NKP, y=PADH,
                                     x=PADW)
            for ch in range(NCHUNK):
                y0 = ch * YCH
                pss = [pp.tile([64, NFREE], mybir.dt.float32,
                               name=f'ps{g}', tag=f'ps{g}')
                       for g in range(4)]
                for p in range(NKP):
                    for d in range(9):
                        dy, dx = d // 3, d % 3
                        m = p * 9 + d
                        g, mi = m // 18, m % 18
                        rhs = gv[:, p, y0 + dy:y0 + dy + YCH, dx:dx + W]
                        nc.tensor.matmul(pss[g][:, :],
                                         ltsb[:, m * 64:(m + 1) * 64],
                                         rhs, start=(mi == 0),
                                         stop=(mi == 17))
                bos = [bp.tile([64, NFREE], mybir.dt.float32,
                               name=f'bo{g}', tag=f'bo{g}')
                       for g in range(4)]
                for g in range(4):
                    nc.vector.tensor_copy(bos[g][:, :], pss[g][:, :])
                nc.vector.tensor_add(bos[0][:, :], bos[0][:, :], bos[1][:, :])
                nc.vector.tensor_add(bos[2][:, :], bos[2][:, :], bos[3][:, :])
                bo = bos[0]
                nc.vector.tensor_add(bo[:, :], bo[:, :], bos[2][:, :])
                nc.sync.dma_start(
                    out=out[:, y0:y0 + YCH, :],
                    in_=bo[:, :].rearrange('p (y x) -> p y x', y=YCH))
    nc.compile()
    return nc


_NC_CACHE = {}


def kernel(vid, ln_w, ln_b, wq, bq, wk, bk, wv, bv, proj_w, proj_b):
    import ml_dtypes
    bf16 = ml_dtypes.bfloat16
    vid = np.asarray(vid, np.float32)
    args = [np.asarray(a, np.float32) for a in
            (ln_w, ln_b, wq, bq, wk, bk, wv, bv)]
    proj_w = np.asarray(proj_w, np.float32)
    proj_b = np.asarray(proj_b, np.float32)

    stack = _host_pre(vid, *args)
    in_maps = _build_core_inputs(stack, proj_w, bf16)

    from concourse.bass_utils import run_bass_kernel_spmd
    if 'nc' not in _NC_CACHE:
        _NC_CACHE['nc'] = _build_bass()
    res = run_bass_kernel_spmd(_NC_CACHE['nc'], in_maps, list(range(8)))

    out = np.zeros((B * T, C, H, W), np.float32)
    for core in range(8):
        t, hp = core // 2, core % 2
        out[t, hp * 64:(hp + 1) * 64] = res.results[core]['out']
    out += proj_b[None, :, None, None]
    return out.reshape(B, T, C, H, W)



# revision 2
# speedup vs baseline: 4.4620x; 4.4620x over previous
"""Full on-device NonLocalAttentionStack kernel for 8 trn2 cores.

Sharding: 8 cores = 4 frames x 2 row-halves (48 out rows each). Per core:
LN -> QKV (PE) -> 49-offset patch search (vector mult + PE ones-reduce +
box sums) -> top-16 via max/max_index/match_replace (pixel-major after PE
transposes) -> softmax (scalar Exp) -> wrapped index build (replicating
transpose-matmuls) -> GPSIMD indirect_copy gather of v (bf16) -> weight
multiply -> grouped Conv3d as 144 PSUM-accumulated block-diag matmuls.
"""
import numpy as np

NH, WS, PS, K = 4, 7, 3, 16
B, T, C, H, W = 1, 4, 128, 96, 96
HD = C // NH                    # 32
SLAB = 58                       # slab rows [-5, 53) around the 48-row half
NS = 50                         # score/stack rows s=0..49 (spatial y = s-1)
WP = 98                         # padded x grid (x = -1..96)
NPX = NS * WP                   # 4900
NTILE = 39
NPXP = NTILE * 128              # 4992
VPC = 102                       # vpad cols (x = -3..98)
VPN = SLAB * VPC                # 5916
EPS_TIE = 1e-5
OUTR = 48
NF = NPXP // 16                 # 312


def _bands():
    b = []
    for y0, y1 in ((0, 24), (24, 48)):
        j_lo = (y0 * WP) // 1024 * 1024
        j_hi = -((-((y1 + 2) * WP)) // 16) * 16
        b.append((j_lo, j_hi - j_lo, j_lo // 16, (j_hi - j_lo) // 16, y0, y1 - y0))
    return b


BANDS = _bands()


def build_nc(debug=False):
    import concourse.bacc as bacc
    import concourse.mybir as mybir
    from concourse.tile import TileContext
    f32, bf16, u16, u32 = (mybir.dt.float32, mybir.dt.bfloat16,
                           mybir.dt.int16, mybir.dt.uint32)
    AF, ALU = mybir.ActivationFunctionType, mybir.AluOpType

    nc = bacc.Bacc()
    din = {}
    for nm, shp, dt in [
        ('vid', [128, SLAB * W], f32), ('wqT', [128, 128], f32),
        ('wkT', [128, 128], f32), ('wvT', [128, 128], f32),
        ('bq', [1, 128], f32), ('bk', [1, 128], f32), ('bv', [1, 128], f32),
        ('valid', [1, SLAB * W], f32), ('cw', [128, 144 * 32], bf16),
        ('ident', [128, 128], f32), ('cs', [128, NTILE], f32),
        ('pm', [128, NTILE], f32), ('eba', [128, 1], f32),
        ('selsrc', [4, 128], bf16),
        ('ebb', [68, 1], f32), ('projb', [128, 1], f32),
    ]:
        din[nm] = nc.declare_dram_parameter(nm, shp, dt, isOutput=False)
    dout = nc.declare_dram_parameter('out', [128, OUTR * W], f32, isOutput=True)
    ddbg = {}
    if debug:
        for nm, shp, dt in [('d_sca', [128, NPXP], f32), ('d_scb', [68, NPXP], f32),
                            ('d_wt', [64, NPXP], bf16), ('d_w16', [128, K * NF], u16),
                            ('d_q', [128, SLAB * W], f32), ('d_vp', [128, VPN], f32),
                            ('d_g', [128, BANDS[1][1]], bf16)]:
            ddbg[nm] = nc.declare_dram_parameter(nm, shp, dt, isOutput=True)

    with TileContext(nc) as tc:
        with (
            tc.tile_pool(name='persist', bufs=1) as PP,
            tc.tile_pool(name='const', bufs=1) as CP,
        ):
            # ---- constants ----
            ID = CP.tile([128, 128], f32, name='ID', tag='ID')
            nc.sync.dma_start(out=ID[:, :], in_=din['ident'][:, :])
            CS = CP.tile([128, NTILE], f32, name='CS', tag='CS')
            nc.sync.dma_start(out=CS[:, :], in_=din['cs'][:, :])
            PM = CP.tile([128, NTILE], f32, name='PM', tag='PM')
            nc.sync.dma_start(out=PM[:, :], in_=din['pm'][:, :])
            EBA = CP.tile([128, 1], f32, name='EBA', tag='EBA')
            nc.sync.dma_start(out=EBA[:, :], in_=din['eba'][:, :])
            EBB = CP.tile([68, 1], f32, name='EBB', tag='EBB')
            nc.sync.dma_start(out=EBB[:, :], in_=din['ebb'][:, :])
            PB = CP.tile([128, 1], f32, name='PB', tag='PB')
            nc.sync.dma_start(out=PB[:, :], in_=din['projb'][:, :])
            WQT = CP.tile([128, 128], f32, name='WQT', tag='WQT')
            nc.sync.dma_start(out=WQT[:, :], in_=din['wqT'][:, :])
            WKT = CP.tile([128, 128], f32, name='WKT', tag='WKT')
            nc.sync.dma_start(out=WKT[:, :], in_=din['wkT'][:, :])
            WVT = CP.tile([128, 128], f32, name='WVT', tag='WVT')
            nc.sync.dma_start(out=WVT[:, :], in_=din['wvT'][:, :])
            BQ = CP.tile([1, 128], f32, name='BQ', tag='BQ')
            nc.sync.dma_start(out=BQ[:, :], in_=din['bq'][:, :])
            BK = CP.tile([1, 128], f32, name='BK', tag='BK')
            nc.sync.dma_start(out=BK[:, :], in_=din['bk'][:, :])
            BV = CP.tile([1, 128], f32, name='BV', tag='BV')
            nc.sync.dma_start(out=BV[:, :], in_=din['bv'][:, :])
            EPSC = CP.tile([1, 1], f32, name='EPSC', tag='EPSC')
            nc.vector.memset(EPSC[:, :], 1e-6)
            ONES1 = CP.tile([1, 128], f32, name='ONES1', tag='ONES1')
            nc.vector.memset(ONES1[:, :], 1.0)
            ONE128 = CP.tile([128, 1], f32, name='ONE128', tag='ONE128')
            nc.vector.memset(ONE128[:, :], 1.0)
            ZB = CP.tile([128, 252], f32, name='ZB', tag='ZB')
            nc.vector.memset(ZB[:, :], 0.0)
            for h in range(4):
                nc.vector.memset(ZB[32 * h:32 * h + 32, 124 + h:125 + h], 1.0)
            SEL = CP.tile([64, K * 128], bf16, name='SEL', tag='SEL')
            nc.vector.memset(SEL[:, :], 0.0)
            for k in range(K):
                nc.sync.dma_start(out=SEL[4 * k:4 * k + 4, 128 * k:128 * k + 128],
                                  in_=din['selsrc'][:, :])

            # ---- persistent data ----
            CL = PP.tile([128, 144 * 128], bf16, name='CL', tag='CL')
            VP = PP.tile([128, VPN], f32, name='VP', tag='VP')
            W16 = PP.tile([128, K * NF], u16, name='W16', tag='W16')
            WT = PP.tile([64, NPXP], bf16, name='WT', tag='WT')
            nc.vector.memset(VP[:, :], 0.0)
            clv = CL[:, :].rearrange('p (m c) -> p m c', c=128)
            vview = VP[:, :].rearrange('p (r c) -> p r c', c=VPC)
            w16v = W16[:, :].rearrange('p (k f) -> p k f', f=NF)

            with tc.tile_pool(name='cwp', bufs=1) as CWP:
                CW = CWP.tile([128, 144 * 32], bf16, name='CW', tag='CW')
                nc.sync.dma_start(out=CW[:, :], in_=din['cw'][:, :])
                nc.vector.memset(CL[:, :], 0.0)
                cwv = CW[:, :].rearrange('p (m c) -> p m c', c=32)
                for m in range(144):
                    for h in range(4):
                        nc.vector.tensor_copy(
                            clv[32 * h:32 * h + 32, m, 32 * h:32 * h + 32],
                            cwv[32 * h:32 * h + 32, m, :])

            # ================= stages 1-3 =================
            with tc.tile_pool(name='sc', bufs=1) as SC:
                SCA = SC.tile([128, NPXP], f32, name='SCA', tag='SCA')
                SCB = SC.tile([68, NPXP], f32, name='SCB', tag='SCB')
                nc.vector.memset(SCA[:, :], 0.0)
                nc.vector.memset(SCB[:, :], 0.0)
                scav = SCA[:, 0:NPX].rearrange('p (s c) -> p s c', c=WP)
                scbv = SCB[:, 0:NPX].rearrange('p (s c) -> p s c', c=WP)

                with tc.tile_pool(name='qk', bufs=1) as QK:
                    Q = QK.tile([128, SLAB * W], f32, name='Q', tag='Q')
                    KP = QK.tile([128, SLAB * VPC], f32, name='KP', tag='KP')
                    nc.vector.memset(KP[:, :], 0.0)
                    kview = KP[:, :].rearrange('p (r c) -> p r c', c=VPC)
                    qview = Q[:, :].rearrange('p (r c) -> p r c', c=W)

                    # ----- stage 1: LN + QKV -----
                    with (
                        tc.tile_pool(name='ln', bufs=2) as LN,
                        tc.tile_pool(name='lnp', bufs=1, space='PSUM') as LNP,
                        tc.tile_pool(name='qkvp', bufs=1, space='PSUM') as QKVP,
                    ):
                        r0 = 0
                        while r0 < SLAB:
                            nr = min(4, SLAB - r0)
                            F = nr * W
                            xc = LN.tile([128, 4 * W], f32, name='xc', tag='xc')
                            nc.sync.dma_start(
                                out=xc[:, 0:F],
                                in_=din['vid'][:, r0 * W:r0 * W + F])
                            vlc = LN.tile([1, 4 * W], f32, name='vlc', tag='vlc')
                            nc.sync.dma_start(
                                out=vlc[:, 0:F],
                                in_=din['valid'][:, r0 * W:r0 * W + F])
                            sq = LN.tile([128, 4 * W], f32, name='sq', tag='sq')
                            nc.scalar.square(sq[:, 0:F], xc[:, 0:F])
                            ps1 = LNP.tile([1, 4 * W], f32, name='ps1', tag='ps1')
                            ps2 = LNP.tile([1, 4 * W], f32, name='ps2', tag='ps2')
                            nc.tensor.matmul(ps1[:, 0:F], ONE128[:, :], xc[:, 0:F],
                                             start=True, stop=True)
                            nc.tensor.matmul(ps2[:, 0:F], ONE128[:, :], sq[:, 0:F],
                                             start=True, stop=True)
                            mu = LN.tile([1, 4 * W], f32, name='mu', tag='mu', bufs=1)
                            nc.vector.tensor_scalar(mu[:, 0:F], ps1[:, 0:F],
                                                    1.0 / 128, None, ALU.mult)
                            var = LN.tile([1, 4 * W], f32, name='var', tag='var', bufs=1)
                            nc.vector.tensor_scalar(var[:, 0:F], ps2[:, 0:F],
                                                    1.0 / 128, None, ALU.mult)
                            mu2 = LN.tile([1, 4 * W], f32, name='mu2', tag='mu2', bufs=1)
                            nc.scalar.square(mu2[:, 0:F], mu[:, 0:F])
                            nc.vector.tensor_tensor(var[:, 0:F], var[:, 0:F],
                                                    mu2[:, 0:F], ALU.subtract)
                            sd = LN.tile([1, 4 * W], f32, name='sd', tag='sd', bufs=1)
                            nc.scalar.activation(sd[:, 0:F], var[:, 0:F], AF.Sqrt,
                                                 bias=EPSC[:, :])
                            rs = LN.tile([1, 4 * W], f32, name='rs', tag='rs', bufs=1)
                            nc.vector.reciprocal(rs[:, 0:F], sd[:, 0:F])
                            pmu = LNP.tile([128, 4 * W], f32, name='pmu', tag='pmu')
                            prs = LNP.tile([128, 4 * W], f32, name='prs', tag='prs')
                            nc.tensor.matmul(pmu[:, 0:F], ONES1[:, :], mu[:, 0:F],
                                             start=True, stop=True)
                            nc.tensor.matmul(prs[:, 0:F], ONES1[:, :], rs[:, 0:F],
                                             start=True, stop=True)
                            xn = LN.tile([128, 4 * W], f32, name='xn', tag='xn')
                            nc.vector.tensor_tensor(xn[:, 0:F], xc[:, 0:F],
                                                    pmu[:, 0:F], ALU.subtract)
                            nc.vector.tensor_tensor(xn[:, 0:F], xn[:, 0:F],
                                                    prs[:, 0:F], ALU.mult)
                            for wt_ap, b_ap, dst in ((WQT, BQ, 'q'), (WKT, BK, 'k'),
                                                     (WVT, BV, 'v')):
                                pq = QKVP.tile([128, 4 * W], f32, name=f'p{dst}',
                                               tag=f'p{dst}')
                                nc.tensor.matmul(pq[:, 0:F], wt_ap[:, :], xn[:, 0:F],
                                                 start=True, stop=False)
                                nc.tensor.matmul(pq[:, 0:F], b_ap[:, :],
                                                 vlc[:, 0:F],
                                                 start=False, stop=True)
                                pqv = pq[:, 0:F].rearrange('p (r c) -> p r c', c=W)
                                if dst == 'q':
                                    nc.vector.tensor_copy(qview[:, r0:r0 + nr, :],
                                                          pqv)
                                elif dst == 'k':
                                    nc.vector.tensor_copy(
                                        kview[:, r0:r0 + nr, 3:99], pqv)
                                else:
                                    nc.vector.tensor_copy(
                                        vview[:, r0:r0 + nr, 3:99], pqv)
                            r0 += nr

                    # ----- stage 2: search + scores -----
                    with (
                        tc.tile_pool(name='pr', bufs=3) as PR,
                        tc.tile_pool(name='ipp', bufs=1, space='PSUM') as IPP,
                        tc.tile_pool(name='xbp', bufs=2) as XBP,
                    ):
                        for s0 in range(0, NS, 3):
                            nr = min(3, NS - s0)
                            ipr = nr + 2
                            F = ipr * W
                            ipA = IPP.tile([128, 5 * W], f32, name='ipA', tag='ipA')
                            ipB = IPP.tile([68, 5 * W], f32, name='ipB', tag='ipB')
                            for o in range(49):
                                dy, dx = o // 7, o % 7
                                P = PR.tile([128, 5 * W], f32, name='P', tag='P')
                                nc.vector.tensor_tensor(
                                    P[:, 0:F].rearrange('p (r c) -> p r c', c=W),
                                    qview[:, s0 + 3:s0 + 3 + ipr, :],
                                    kview[:, s0 + dy:s0 + dy + ipr, dx:dx + W],
                                    ALU.mult)
                                if o < 32:
                                    nc.tensor.matmul(
                                        ipA[:, 0:F], ZB[:, 124 - 4 * o:252 - 4 * o],
                                        P[:, 0:F], start=(o == 0), stop=(o == 31))
                                else:
                                    o2 = o - 32
                                    nc.tensor.matmul(
                                        ipB[:, 0:F],
                                        ZB[:, 124 - 4 * o2:192 - 4 * o2],
                                        P[:, 0:F], start=(o == 32), stop=(o == 48))
                            for (ip, scv, nprt, eb) in ((ipA, scav, 128, EBA),
                                                        (ipB, scbv, 68, EBB)):
                                ips = XBP.tile([128, 5 * W], f32, name='ips',
                                               tag='ips')
                                nc.vector.tensor_copy(ips[0:nprt, 0:F],
                                                      ip[0:nprt, 0:F])
                                ipv = ips[0:nprt, 0:F].rearrange('p (r c) -> p r c',
                                                                 c=W)
                                xb = XBP.tile([128, 5 * WP], f32, name='xb', tag='xb')
                                xbv = xb[0:nprt, 0:ipr * WP].rearrange(
                                    'p (r c) -> p r c', c=WP)
                                nc.vector.tensor_tensor(xbv[:, :, 2:96],
                                                        ipv[:, :, 0:94],
                                                        ipv[:, :, 1:95], ALU.add)
                                nc.vector.tensor_tensor(xbv[:, :, 2:96],
                                                        xbv[:, :, 2:96],
                                                        ipv[:, :, 2:96], ALU.add)
                                nc.vector.tensor_tensor(xbv[:, :, 1:2],
                                                        ipv[:, :, 0:1],
                                                        ipv[:, :, 1:2], ALU.add)
                                nc.vector.tensor_tensor(xbv[:, :, 96:97],
                                                        ipv[:, :, 94:95],
                                                        ipv[:, :, 95:96], ALU.add)
                                nc.vector.tensor_tensor(scv[0:nprt, s0:s0 + nr, 1:97],
                                                        xbv[:, 0:nr, 1:97],
                                                        xbv[:, 1:nr + 1, 1:97],
                                                        ALU.add)
                                nc.vector.tensor_tensor(scv[0:nprt, s0:s0 + nr, 1:97],
                                                        scv[0:nprt, s0:s0 + nr, 1:97],
                                                        xbv[:, 2:nr + 2, 1:97],
                                                        ALU.add)
                                nc.vector.tensor_scalar(scv[0:nprt, s0:s0 + nr, 1:97],
                                                        scv[0:nprt, s0:s0 + nr, 1:97],
                                                        eb[0:nprt, :], None, ALU.add)
                    if debug:
                        nc.sync.dma_start(out=ddbg['d_sca'][:, :], in_=SCA[:, :])
                        nc.sync.dma_start(out=ddbg['d_scb'][:, :], in_=SCB[:, :])
                        nc.sync.dma_start(out=ddbg['d_q'][:, :], in_=Q[:, :])

                # ----- stage 3: transpose + topk + softmax + wrapped idx -----
                with (
                    tc.tile_pool(name='tk', bufs=2) as TK,
                    tc.tile_pool(name='tkp', bufs=1, space='PSUM') as TKP,
                    tc.tile_pool(name='w16p', bufs=2, space='PSUM') as W16P,
                ):
                    for ci in range(NTILE):
                        c0 = 128 * ci
                        T1 = TKP.tile([128, 128], f32, name='T1', tag='T1')
                        nc.tensor.transpose(T1[:, :], SCA[:, c0:c0 + 128], ID[:, :])
                        T2 = TKP.tile([128, 68], f32, name='T2', tag='T2')
                        nc.tensor.transpose(T2[:, :], SCB[:, c0:c0 + 128],
                                            ID[0:68, 0:68])
                        S = TK.tile([128, 196], f32, name='S', tag='S')
                        nc.vector.tensor_copy(S[:, 0:128], T1[:, :])
                        nc.vector.tensor_copy(S[:, 128:196], T2[:, :])
                        IW = TK.tile([128, 128], f32, name='IW', tag='IW')
                        for h in range(4):
                            hv = S[:, :].rearrange('p (o h) -> p h o', h=4)[:, h, :]
                            m1 = TK.tile([128, 8], f32, name='m1', tag='m1')
                            m2 = TK.tile([128, 8], f32, name='m2', tag='m2')
                            i1 = TK.tile([128, 8], u32, name='i1', tag='i1')
                            i2 = TK.tile([128, 8], u32, name='i2', tag='i2')
                            nc.vector.max(m1[:, :], hv)
                            nc.vector.max_index(i1[:, :], m1[:, :], hv)
                            nc.vector.match_replace(hv, m1[:, :], hv, -1e30)
                            nc.vector.max(m2[:, :], hv)
                            nc.vector.max_index(i2[:, :], m2[:, :], hv)
                            iwi = IW[:, 0:64].rearrange('p (k h) -> p h k',
                                                        h=4)[:, h, :]
                            nc.vector.tensor_copy(iwi[:, 0:8], i1[:, :])
                            nc.vector.tensor_copy(iwi[:, 8:16], i2[:, :])
                            nv = TK.tile([128, 1], f32, name='nv', tag='nv')
                            nc.vector.tensor_scalar(nv[:, :], m1[:, 0:1], -1.0,
                                                    None, ALU.mult)
                            iww = IW[:, 64:128].rearrange('p (k h) -> p h k',
                                                          h=4)[:, h, :]
                            s1 = TK.tile([128, 1], f32, name='s1', tag='s1')
                            s2 = TK.tile([128, 1], f32, name='s2', tag='s2')
                            nc.scalar.activation(iww[:, 0:8], m1[:, :], AF.Exp,
                                                 bias=nv[:, :], accum_out=s1[:, :])
                            nc.scalar.activation(iww[:, 8:16], m2[:, :], AF.Exp,
                                                 bias=nv[:, :], accum_out=s2[:, :])
                            nc.vector.tensor_tensor(s1[:, :], s1[:, :], s2[:, :],
                                                    ALU.add)
                            rc = TK.tile([128, 1], f32, name='rc', tag='rc')
                            nc.vector.reciprocal(rc[:, :], s1[:, :])
                            nc.vector.tensor_scalar(iww[:, :], iww[:, :], rc[:, :],
                                                    None, ALU.mult)
                        t1 = TK.tile([128, 64], f32, name='t1', tag='t1')
                        t2 = TK.tile([128, 64], f32, name='t2', tag='t2')
                        # lin = C + idx + 95*floor(idx/7); floor via >= ladder
                        nc.vector.tensor_scalar(t1[:, :], IW[:, 0:64], 6.5, 95.0,
                                                ALU.is_gt, ALU.mult)
                        for m in range(2, 7):
                            nc.vector.tensor_scalar(t2[:, :], IW[:, 0:64],
                                                    7.0 * m - 0.5, 95.0,
                                                    ALU.is_gt, ALU.mult)
                            nc.vector.tensor_tensor(t1[:, :], t1[:, :], t2[:, :],
                                                    ALU.add)
                        nc.vector.tensor_tensor(t1[:, :], t1[:, :], IW[:, 0:64],
                                                ALU.add)
                        nc.vector.tensor_scalar(IW[:, 0:64], t1[:, :],
                                                CS[:, ci:ci + 1], None, ALU.add)
                        nc.vector.tensor_scalar(IW[:, 64:128], IW[:, 64:128],
                                                PM[:, ci:ci + 1], None, ALU.mult)
                        TIW = TKP.tile([128, 128], f32, name='TIW', tag='TIW')
                        nc.tensor.transpose(TIW[:, :], IW[:, :], ID[:, :])
                        ITS = TK.tile([64, 128], f32, name='ITS', tag='ITS')
                        nc.vector.tensor_copy(ITS[:, :], TIW[0:64, :])
                        nc.vector.tensor_copy(WT[:, c0:c0 + 128], TIW[64:128, :])
                        PW = W16P.tile([128, 512], f32, name='PW', tag='PW')
                        for f in range(8):
                            IR = TK.tile([64, 128], f32, name='IR', tag='IR')
                            nc.vector.tensor_copy(IR[:, 0:16],
                                                  ITS[:, 16 * f:16 * f + 16])
                            nc.vector.tensor_copy(IR[:, 16:32], IR[:, 0:16])
                            nc.vector.tensor_copy(IR[:, 32:64], IR[:, 0:32])
                            nc.vector.tensor_copy(IR[:, 64:128], IR[:, 0:64])
                            nc.tensor.matmul(PW[:, 64 * f:64 * f + 64], IR[:, :],
                                             ID[0:64, 0:64], start=True, stop=True)
                        pwv = PW[:, :].rearrange('p (f c) -> p f c', c=64)
                        for h in range(4):
                            src = pwv[32 * h:32 * h + 32, :, :].rearrange(
                                'p f (k h2) -> p h2 k f', h2=4)[:, h, :, :]
                            nc.vector.tensor_copy(
                                w16v[32 * h:32 * h + 32, :, 8 * ci:8 * ci + 8], src)
                    if debug:
                        nc.sync.dma_start(out=ddbg['d_wt'][:, :], in_=WT[:, :])
                        nc.sync.dma_start(out=ddbg['d_w16'][:, :], in_=W16[:, :])
                        nc.sync.dma_start(out=ddbg['d_vp'][:, :], in_=VP[:, :])

            # ================= stage 4: gather + conv =================
            with (
                tc.tile_pool(name='g', bufs=1) as GP,
                tc.tile_pool(name='cv', bufs=2) as CV,
                tc.tile_pool(name='cvp', bufs=2, space='PSUM') as CVP,
            ):
                outv = dout[:, :].rearrange('p (y c) -> p y c', c=W)
                Lmax = max(bd[1] for bd in BANDS)
                for bi, (j0, L, f0, Fb, y0, nry) in reversed(list(enumerate(BANDS))):
                    Gs = []
                    for k in range(K):
                        G = GP.tile([128, Lmax], bf16, name=f'G{k}', tag=f'G{k}')
                        for c0 in range(0, L, 1024):
                            w = min(1024, L - c0)
                            GT = CV.tile([128, 1024], f32, name='GT', tag='GT')
                            nc.gpsimd.ap_gather(
                                GT[:, 0:w], VP[:, :],
                                w16v[:, k, f0 + c0 // 16:f0 + (c0 + w) // 16],
                                channels=128, num_elems=VPN, d=1, num_idxs=w)
                            for s0 in range(0, w, 512):
                                sw = min(512, w - s0)
                                PWB = CVP.tile([128, 512], f32, name='PWB',
                                               tag='PWB')
                                nc.tensor.matmul(
                                    PWB[:, 0:sw], SEL[:, 128 * k:128 * k + 128],
                                    WT[:, j0 + c0 + s0:j0 + c0 + s0 + sw],
                                    start=True, stop=True)
                                nc.vector.tensor_tensor(GT[:, s0:s0 + sw],
                                                        GT[:, s0:s0 + sw],
                                                        PWB[:, 0:sw], ALU.mult)
                            nc.vector.tensor_copy(G[:, c0:c0 + w], GT[:, 0:w])
                        Gs.append(G)
                    if debug and bi == 1:
                        nc.sync.dma_start(out=ddbg['d_g'][:, :],
                                          in_=Gs[0][:, 0:BANDS[1][1]])
                    yy = 0
                    while yy < nry:
                        cr = min(5, nry - yy)
                        F = cr * W
                        PO = CVP.tile([128, 5 * W], f32, name='PO', tag='PO')
                        n = 0
                        for k in range(K):
                            for d in range(9):
                                dy, dx = d // 3, d % 3
                                off = (y0 + yy + dy) * WP + dx - j0
                                rhs = Gs[k][:, off:off + cr * WP].rearrange(
                                    'p (r c) -> p r c', c=WP)[:, :, 0:W]
                                nc.tensor.matmul(PO[:, 0:F], clv[:, 9 * k + d, :],
                                                 rhs, start=(n == 0), stop=(n == 143))
                                n += 1
                        OSB = CV.tile([128, 5 * W], f32, name='OSB', tag='OSB')
                        nc.vector.tensor_scalar(OSB[:, 0:F], PO[:, 0:F], PB[:, :],
                                                None, ALU.add)
                        nc.sync.dma_start(
                            out=outv[:, y0 + yy:y0 + yy + cr, :],
                            in_=OSB[:, 0:F].rearrange('p (r c) -> p r c', c=W))
                        yy += cr
    nc.compile()
    return nc


# ======================= host side =======================

def host_inputs(vid, ln_w, ln_b, wq, bq, wk, bk, wv, bv, proj_w, proj_b):
    """Build the 8 per-core input dicts."""
    import ml_dtypes
    bf = ml_dtypes.bfloat16
    vid = np.asarray(vid, np.float32)

    def prep_w(w, b):
        wp = (np.asarray(w, np.float32) * np.asarray(ln_w, np.float32)[None, :])
        beta = np.asarray(w, np.float32) @ np.asarray(ln_b, np.float32) + \
            np.asarray(b, np.float32)
        return np.ascontiguousarray(wp.T), beta.reshape(1, 128)

    wqT, bqr = prep_w(wq, bq)
    wkT, bkr = prep_w(wk, bk)
    wvT, bvr = prep_w(wv, bv)

    pw = np.asarray(proj_w, np.float32)          # (128, 32, 16, 3, 3)
    cw = np.zeros((128, 144, 32), np.float32)
    for h in range(4):
        for i in range(HD):
            for k in range(K):
                for d in range(9):
                    cw[32 * h + i, 9 * k + d, :] = pw[32 * h:32 * h + 32, i, k,
                                                      d // 3, d % 3]
    cw = cw.reshape(128, 144 * 32).astype(bf)

    ident = np.eye(128, dtype=np.float32)
    cs = np.zeros((128, NTILE), np.float32)
    pms = [np.zeros((128, NTILE), np.float32) for _ in range(2)]
    for ci in range(NTILE):
        for p in range(128):
            j = 128 * ci + p
            s, xp = j // WP, j % WP
            if j < NPX and 1 <= xp <= 96:
                cs[p, ci] = (s + 1) * VPC + (xp - 1) + 0.25
                for half in range(2):
                    if 0 <= 48 * half + s - 1 < H:
                        pms[half][p, ci] = 1.0
    eba = np.array([[-EPS_TIE * (p // 4)] for p in range(128)], np.float32)
    selsrc = np.zeros((4, 128), np.float32)
    for h in range(4):
        selsrc[h, 32 * h:32 * h + 32] = 1.0
    selsrc = selsrc.astype(bf)
    ebb = np.array([[-EPS_TIE * (32 + p // 4)] for p in range(68)], np.float32)
    pbr = np.asarray(proj_b, np.float32).reshape(128, 1)

    maps = []
    for core in range(8):
        t, half = core // 2, core % 2
        y0 = 48 * half
        slab = np.zeros((SLAB, 128, W), np.float32)
        valid = np.zeros((SLAB, W), np.float32)
        for r in range(SLAB):
            g = y0 - 5 + r
            if 0 <= g < H:
                slab[r] = vid[0, t, :, g, :]
                valid[r] = 1.0
        maps.append({
            'vid': np.ascontiguousarray(slab.transpose(1, 0, 2)).reshape(
                128, SLAB * W),
            'wqT': wqT, 'wkT': wkT, 'wvT': wvT,
            'bq': bqr, 'bk': bkr, 'bv': bvr,
            'valid': valid.reshape(1, SLAB * W),
            'cw': cw, 'ident': ident, 'cs': cs, 'pm': pms[half],
            'selsrc': selsrc,
            'eba': eba, 'ebb': ebb, 'projb': pbr,
        })
    return maps


_CACHE = {}


def kernel(vid, ln_w, ln_b, wq, bq, wk, bk, wv, bv, proj_w, proj_b):
    from concourse.bass_utils import run_bass_kernel_spmd
    maps = host_inputs(vid, ln_w, ln_b, wq, bq, wk, bk, wv, bv, proj_w, proj_b)
    if 'nc' not in _CACHE:
        _CACHE['nc'] = build_nc()
    res = run_bass_kernel_spmd(_CACHE['nc'], maps, list(range(8)))
    out = np.zeros((T, C, H, W), np.float32)
    for core in range(8):
        t, half = core // 2, core % 2
        out[t, :, 48 * half:48 * half + 48, :] = \
            res.results[core]['out'].reshape(128, 48, W)
    return out.reshape(B, T, C, H, W)


# revision 4
# speedup vs baseline: 4.5884x; 1.0283x over previous
"""Full on-device NonLocalAttentionStack kernel for 8 trn2 cores.

Sharding: 8 cores = 4 frames x 2 row-halves (48 out rows each). Per core:
LN -> QKV (PE) -> 49-offset patch search (vector mult + PE ones-reduce +
box sums) -> top-16 via max/max_index/match_replace (pixel-major after PE
transposes) -> softmax (scalar Exp) -> wrapped index build (replicating
transpose-matmuls) -> GPSIMD indirect_copy gather of v (bf16) -> weight
multiply -> grouped Conv3d as 144 PSUM-accumulated block-diag matmuls.
"""
import numpy as np

NH, WS, PS, K = 4, 7, 3, 16
B, T, C, H, W = 1, 4, 128, 96, 96
HD = C // NH                    # 32
SLAB = 58                       # slab rows [-5, 53) around the 48-row half
NS = 50                         # score/stack rows s=0..49 (spatial y = s-1)
WP = 98                         # padded x grid (x = -1..96)
NPX = NS * WP                   # 4900
NTILE = 39
NPXP = NTILE * 128              # 4992
VPC = 102                       # vpad cols (x = -3..98)
VPN = SLAB * VPC                # 5916
EPS_TIE = 1e-5
OUTR = 48
NF = NPXP // 16                 # 312


def _bands():
    b = []
    for y0, y1 in ((0, 24), (24, 48)):
        j_lo = (y0 * WP) // 1024 * 1024
        j_hi = -((-((y1 + 2) * WP)) // 16) * 16
        b.append((j_lo, j_hi - j_lo, j_lo // 16, (j_hi - j_lo) // 16, y0, y1 - y0))
    return b


BANDS = _bands()


def build_nc(debug=False):
    import concourse.bacc as bacc
    import concourse.mybir as mybir
    from concourse.tile import TileContext
    f32, bf16, u16, u32 = (mybir.dt.float32, mybir.dt.bfloat16,
                           mybir.dt.int16, mybir.dt.uint32)
    AF, ALU = mybir.ActivationFunctionType, mybir.AluOpType

    nc = bacc.Bacc()
    din = {}
    for nm, shp, dt in [
        ('vid', [128, SLAB * W], f32), ('wqT', [128, 128], f32),
        ('wkT', [128, 128], f32), ('wvT', [128, 128], f32),
        ('bq', [1, 128], f32), ('bk', [1, 128], f32), ('bv', [1, 128], f32),
        ('valid', [1, SLAB * W], f32), ('cw', [128, 144 * 32], bf16),
        ('ident', [128, 128], f32), ('cs', [128, NTILE], f32),
        ('pm', [128, NTILE], f32), ('eba', [128, 1], f32),
        ('selsrc', [4, 128], bf16),
        ('ebb', [68, 1], f32), ('projb', [128, 1], f32),
    ]:
        din[nm] = nc.declare_dram_parameter(nm, shp, dt, isOutput=False)
    dout = nc.declare_dram_parameter('out', [128, OUTR * W], f32, isOutput=True)
    ddbg = {}
    if debug:
        for nm, shp, dt in [('d_sca', [128, NPXP], f32), ('d_scb', [68, NPXP], f32),
                            ('d_wt', [64, NPXP], bf16), ('d_w16', [128, K * NF], u16),
                            ('d_q', [128, SLAB * W], f32), ('d_vp', [128, VPN], f32),
                            ('d_g', [128, BANDS[1][1]], bf16)]:
            ddbg[nm] = nc.declare_dram_parameter(nm, shp, dt, isOutput=True)

    with TileContext(nc) as tc:
        with (
            tc.tile_pool(name='persist', bufs=1) as PP,
            tc.tile_pool(name='const', bufs=1) as CP,
        ):
            # ---- constants ----
            ID = CP.tile([128, 128], f32, name='ID', tag='ID')
            nc.sync.dma_start(out=ID[:, :], in_=din['ident'][:, :])
            CS = CP.tile([128, NTILE], f32, name='CS', tag='CS')
            nc.sync.dma_start(out=CS[:, :], in_=din['cs'][:, :])
            PM = CP.tile([128, NTILE], f32, name='PM', tag='PM')
            nc.sync.dma_start(out=PM[:, :], in_=din['pm'][:, :])
            EBA = CP.tile([128, 1], f32, name='EBA', tag='EBA')
            nc.sync.dma_start(out=EBA[:, :], in_=din['eba'][:, :])
            EBB = CP.tile([68, 1], f32, name='EBB', tag='EBB')
            nc.sync.dma_start(out=EBB[:, :], in_=din['ebb'][:, :])
            PB = CP.tile([128, 1], f32, name='PB', tag='PB')
            nc.sync.dma_start(out=PB[:, :], in_=din['projb'][:, :])
            WQT = CP.tile([128, 128], f32, name='WQT', tag='WQT')
            nc.sync.dma_start(out=WQT[:, :], in_=din['wqT'][:, :])
            WKT = CP.tile([128, 128], f32, name='WKT', tag='WKT')
            nc.sync.dma_start(out=WKT[:, :], in_=din['wkT'][:, :])
            WVT = CP.tile([128, 128], f32, name='WVT', tag='WVT')
            nc.sync.dma_start(out=WVT[:, :], in_=din['wvT'][:, :])
            BQ = CP.tile([1, 128], f32, name='BQ', tag='BQ')
            nc.sync.dma_start(out=BQ[:, :], in_=din['bq'][:, :])
            BK = CP.tile([1, 128], f32, name='BK', tag='BK')
            nc.sync.dma_start(out=BK[:, :], in_=din['bk'][:, :])
            BV = CP.tile([1, 128], f32, name='BV', tag='BV')
            nc.sync.dma_start(out=BV[:, :], in_=din['bv'][:, :])
            EPSC = CP.tile([1, 1], f32, name='EPSC', tag='EPSC')
            nc.vector.memset(EPSC[:, :], 1e-6)
            ONES1 = CP.tile([1, 128], f32, name='ONES1', tag='ONES1')
            nc.vector.memset(ONES1[:, :], 1.0)
            ONE128 = CP.tile([128, 1], f32, name='ONE128', tag='ONE128')
            nc.vector.memset(ONE128[:, :], 1.0)
            ZB = CP.tile([128, 252], f32, name='ZB', tag='ZB')
            nc.vector.memset(ZB[:, :], 0.0)
            for h in range(4):
                nc.vector.memset(ZB[32 * h:32 * h + 32, 124 + h:125 + h], 1.0)
            SEL = CP.tile([64, K * 128], bf16, name='SEL', tag='SEL')
            nc.vector.memset(SEL[:, :], 0.0)
            for k in range(K):
                nc.sync.dma_start(out=SEL[4 * k:4 * k + 4, 128 * k:128 * k + 128],
                                  in_=din['selsrc'][:, :])

            # ---- persistent data ----
            CL = PP.tile([128, 144 * 128], bf16, name='CL', tag='CL')
            VP = PP.tile([128, VPN], f32, name='VP', tag='VP')
            W16 = PP.tile([128, K * NF], u16, name='W16', tag='W16')
            WT = PP.tile([64, NPXP], bf16, name='WT', tag='WT')
            nc.vector.memset(VP[:, :], 0.0)
            clv = CL[:, :].rearrange('p (m c) -> p m c', c=128)
            vview = VP[:, :].rearrange('p (r c) -> p r c', c=VPC)
            w16v = W16[:, :].rearrange('p (k f) -> p k f', f=NF)

            with tc.tile_pool(name='cwp', bufs=1) as CWP:
                CW = CWP.tile([128, 144 * 32], bf16, name='CW', tag='CW')
                nc.sync.dma_start(out=CW[:, :], in_=din['cw'][:, :])
                nc.vector.memset(CL[:, :], 0.0)
                cwv = CW[:, :].rearrange('p (m c) -> p m c', c=32)
                for m in range(144):
                    for h in range(4):
                        nc.vector.tensor_copy(
                            clv[32 * h:32 * h + 32, m, 32 * h:32 * h + 32],
                            cwv[32 * h:32 * h + 32, m, :])

            # ================= stages 1-3 =================
            with tc.tile_pool(name='sc', bufs=1) as SC:
                SCA = SC.tile([128, NPXP], f32, name='SCA', tag='SCA')
                SCB = SC.tile([68, NPXP], f32, name='SCB', tag='SCB')
                nc.vector.memset(SCA[:, :], 0.0)
                nc.vector.memset(SCB[:, :], 0.0)
                scav = SCA[:, 0:NPX].rearrange('p (s c) -> p s c', c=WP)
                scbv = SCB[:, 0:NPX].rearrange('p (s c) -> p s c', c=WP)

                with tc.tile_pool(name='qk', bufs=1) as QK:
                    Q = QK.tile([128, SLAB * W], f32, name='Q', tag='Q')
                    KP = QK.tile([128, SLAB * VPC], f32, name='KP', tag='KP')
                    nc.vector.memset(KP[:, :], 0.0)
                    kview = KP[:, :].rearrange('p (r c) -> p r c', c=VPC)
                    qview = Q[:, :].rearrange('p (r c) -> p r c', c=W)

                    # ----- stage 1: LN + QKV -----
                    with (
                        tc.tile_pool(name='ln', bufs=2) as LN,
                        tc.tile_pool(name='lnp', bufs=1, space='PSUM') as LNP,
                        tc.tile_pool(name='qkvp', bufs=1, space='PSUM') as QKVP,
                    ):
                        r0 = 0
                        while r0 < SLAB:
                            nr = min(4, SLAB - r0)
                            F = nr * W
                            xc = LN.tile([128, 4 * W], f32, name='xc', tag='xc')
                            nc.sync.dma_start(
                                out=xc[:, 0:F],
                                in_=din['vid'][:, r0 * W:r0 * W + F])
                            vlc = LN.tile([1, 4 * W], f32, name='vlc', tag='vlc')
                            nc.sync.dma_start(
                                out=vlc[:, 0:F],
                                in_=din['valid'][:, r0 * W:r0 * W + F])
                            sq = LN.tile([128, 4 * W], f32, name='sq', tag='sq')
                            nc.scalar.square(sq[:, 0:F], xc[:, 0:F])
                            ps1 = LNP.tile([1, 4 * W], f32, name='ps1', tag='ps1')
                            ps2 = LNP.tile([1, 4 * W], f32, name='ps2', tag='ps2')
                            nc.tensor.matmul(ps1[:, 0:F], ONE128[:, :], xc[:, 0:F],
                                             start=True, stop=True)
                            nc.tensor.matmul(ps2[:, 0:F], ONE128[:, :], sq[:, 0:F],
                                             start=True, stop=True)
                            mu = LN.tile([1, 4 * W], f32, name='mu', tag='mu', bufs=1)
                            nc.vector.tensor_scalar(mu[:, 0:F], ps1[:, 0:F],
                                                    1.0 / 128, None, ALU.mult)
                            var = LN.tile([1, 4 * W], f32, name='var', tag='var', bufs=1)
                            nc.vector.tensor_scalar(var[:, 0:F], ps2[:, 0:F],
                                                    1.0 / 128, None, ALU.mult)
                            mu2 = LN.tile([1, 4 * W], f32, name='mu2', tag='mu2', bufs=1)
                            nc.scalar.square(mu2[:, 0:F], mu[:, 0:F])
                            nc.vector.tensor_tensor(var[:, 0:F], var[:, 0:F],
                                                    mu2[:, 0:F], ALU.subtract)
                            sd = LN.tile([1, 4 * W], f32, name='sd', tag='sd', bufs=1)
                            nc.scalar.activation(sd[:, 0:F], var[:, 0:F], AF.Sqrt,
                                                 bias=EPSC[:, :])
                            rs = LN.tile([1, 4 * W], f32, name='rs', tag='rs', bufs=1)
                            nc.vector.reciprocal(rs[:, 0:F], sd[:, 0:F])
                            pmu = LNP.tile([128, 4 * W], f32, name='pmu', tag='pmu')
                            prs = LNP.tile([128, 4 * W], f32, name='prs', tag='prs')
                            nc.tensor.matmul(pmu[:, 0:F], ONES1[:, :], mu[:, 0:F],
                                             start=True, stop=True)
                            nc.tensor.matmul(prs[:, 0:F], ONES1[:, :], rs[:, 0:F],
                                             start=True, stop=True)
                            xn = LN.tile([128, 4 * W], f32, name='xn', tag='xn')
                            nc.vector.tensor_tensor(xn[:, 0:F], xc[:, 0:F],
                                                    pmu[:, 0:F], ALU.subtract)
                            nc.vector.tensor_tensor(xn[:, 0:F], xn[:, 0:F],
                                                    prs[:, 0:F], ALU.mult)
                            for wt_ap, b_ap, dst in ((WQT, BQ, 'q'), (WKT, BK, 'k'),
                                                     (WVT, BV, 'v')):
                                pq = QKVP.tile([128, 4 * W], f32, name=f'p{dst}',
                                               tag=f'p{dst}')
                                nc.tensor.matmul(pq[:, 0:F], wt_ap[:, :], xn[:, 0:F],
                                                 start=True, stop=False)
                                nc.tensor.matmul(pq[:, 0:F], b_ap[:, :],
                                                 vlc[:, 0:F],
                                                 start=False, stop=True)
                                pqv = pq[:, 0:F].rearrange('p (r c) -> p r c', c=W)
                                if dst == 'q':
                                    nc.vector.tensor_copy(qview[:, r0:r0 + nr, :],
                                                          pqv)
                                elif dst == 'k':
                                    nc.vector.tensor_copy(
                                        kview[:, r0:r0 + nr, 3:99], pqv)
                                else:
                                    nc.vector.tensor_copy(
                                        vview[:, r0:r0 + nr, 3:99], pqv)
                            r0 += nr

                    # ----- stage 2: search + scores -----
                    with (
                        tc.tile_pool(name='pr', bufs=3) as PR,
                        tc.tile_pool(name='ipp', bufs=1, space='PSUM') as IPP,
                        tc.tile_pool(name='xbp', bufs=2) as XBP,
                    ):
                        for s0 in range(0, NS, 3):
                            nr = min(3, NS - s0)
                            ipr = nr + 2
                            F = ipr * W
                            ipA = IPP.tile([128, 5 * W], f32, name='ipA', tag='ipA')
                            ipB = IPP.tile([68, 5 * W], f32, name='ipB', tag='ipB')
                            for o in range(49):
                                dy, dx = o // 7, o % 7
                                P = PR.tile([128, 5 * W], f32, name='P', tag='P')
                                nc.vector.tensor_tensor(
                                    P[:, 0:F].rearrange('p (r c) -> p r c', c=W),
                                    qview[:, s0 + 3:s0 + 3 + ipr, :],
                                    kview[:, s0 + dy:s0 + dy + ipr, dx:dx + W],
                                    ALU.mult)
                                if o < 32:
                                    nc.tensor.matmul(
                                        ipA[:, 0:F], ZB[:, 124 - 4 * o:252 - 4 * o],
                                        P[:, 0:F], start=(o == 0), stop=(o == 31))
                                else:
                                    o2 = o - 32
                                    nc.tensor.matmul(
                                        ipB[:, 0:F],
                                        ZB[:, 124 - 4 * o2:192 - 4 * o2],
                                        P[:, 0:F], start=(o == 32), stop=(o == 48))
                            for (ip, scv, nprt, eb) in ((ipA, scav, 128, EBA),
                                                        (ipB, scbv, 68, EBB)):
                                ips = XBP.tile([128, 5 * W], f32, name='ips',
                                               tag='ips')
                                nc.vector.tensor_copy(ips[0:nprt, 0:F],
                                                      ip[0:nprt, 0:F])
                                ipv = ips[0:nprt, 0:F].rearrange('p (r c) -> p r c',
                                                                 c=W)
                                xb = XBP.tile([128, 5 * WP], f32, name='xb', tag='xb')
                                xbv = xb[0:nprt, 0:ipr * WP].rearrange(
                                    'p (r c) -> p r c', c=WP)
                                nc.vector.tensor_tensor(xbv[:, :, 2:96],
                                                        ipv[:, :, 0:94],
                                                        ipv[:, :, 1:95], ALU.add)
                                nc.vector.tensor_tensor(xbv[:, :, 2:96],
                                                        xbv[:, :, 2:96],
                                                        ipv[:, :, 2:96], ALU.add)
                                nc.vector.tensor_tensor(xbv[:, :, 1:2],
                                                        ipv[:, :, 0:1],
                                                        ipv[:, :, 1:2], ALU.add)
                                nc.vector.tensor_tensor(xbv[:, :, 96:97],
                                                        ipv[:, :, 94:95],
                                                        ipv[:, :, 95:96], ALU.add)
                                nc.vector.tensor_tensor(scv[0:nprt, s0:s0 + nr, 1:97],
                                                        xbv[:, 0:nr, 1:97],
                                                        xbv[:, 1:nr + 1, 1:97],
                                                        ALU.add)
                                nc.vector.tensor_tensor(scv[0:nprt, s0:s0 + nr, 1:97],
                                                        scv[0:nprt, s0:s0 + nr, 1:97],
                                                        xbv[:, 2:nr + 2, 1:97],
                                                        ALU.add)
                                nc.vector.tensor_scalar(scv[0:nprt, s0:s0 + nr, 1:97],
                                                        scv[0:nprt, s0:s0 + nr, 1:97],
                                                        eb[0:nprt, :], None, ALU.add)
                    if debug:
                        nc.sync.dma_start(out=ddbg['d_sca'][:, :], in_=SCA[:, :])
                        nc.sync.dma_start(out=ddbg['d_scb'][:, :], in_=SCB[:, :])
                        nc.sync.dma_start(out=ddbg['d_q'][:, :], in_=Q[:, :])

                # ----- stage 3: transpose + topk + softmax + wrapped idx -----
                with (
                    tc.tile_pool(name='tk', bufs=2) as TK,
                    tc.tile_pool(name='tkp', bufs=1, space='PSUM') as TKP,
                    tc.tile_pool(name='w16p', bufs=2, space='PSUM') as W16P,
                ):
                    for ci in range(NTILE):
                        c0 = 128 * ci
                        T1 = TKP.tile([128, 128], f32, name='T1', tag='T1')
                        nc.tensor.transpose(T1[:, :], SCA[:, c0:c0 + 128], ID[:, :])
                        T2 = TKP.tile([128, 68], f32, name='T2', tag='T2')
                        nc.tensor.transpose(T2[:, :], SCB[:, c0:c0 + 128],
                                            ID[0:68, 0:68])
                        S = TK.tile([128, 196], f32, name='S', tag='S')
                        nc.vector.tensor_copy(S[:, 0:128], T1[:, :])
                        nc.vector.tensor_copy(S[:, 128:196], T2[:, :])
                        IW = TK.tile([128, 128], f32, name='IW', tag='IW')
                        for h in range(4):
                            hv = S[:, :].rearrange('p (o h) -> p h o', h=4)[:, h, :]
                            m1 = TK.tile([128, 8], f32, name='m1', tag='m1')
                            m2 = TK.tile([128, 8], f32, name='m2', tag='m2')
                            i1 = TK.tile([128, 8], u32, name='i1', tag='i1')
                            i2 = TK.tile([128, 8], u32, name='i2', tag='i2')
                            nc.vector.max(m1[:, :], hv)
                            nc.vector.max_index(i1[:, :], m1[:, :], hv)
                            nc.vector.match_replace(hv, m1[:, :], hv, -1e30)
                            nc.vector.max(m2[:, :], hv)
                            nc.vector.max_index(i2[:, :], m2[:, :], hv)
                            iwi = IW[:, 0:64].rearrange('p (k h) -> p h k',
                                                        h=4)[:, h, :]
                            nc.vector.tensor_copy(iwi[:, 0:8], i1[:, :])
                            nc.vector.tensor_copy(iwi[:, 8:16], i2[:, :])
                            nv = TK.tile([128, 1], f32, name='nv', tag='nv')
                            nc.vector.tensor_scalar(nv[:, :], m1[:, 0:1], -1.0,
                                                    None, ALU.mult)
                            iww = IW[:, 64:128].rearrange('p (k h) -> p h k',
                                                          h=4)[:, h, :]
                            s1 = TK.tile([128, 1], f32, name='s1', tag='s1')
                            s2 = TK.tile([128, 1], f32, name='s2', tag='s2')
                            nc.scalar.activation(iww[:, 0:8], m1[:, :], AF.Exp,
                                                 bias=nv[:, :], accum_out=s1[:, :])
                            nc.scalar.activation(iww[:, 8:16], m2[:, :], AF.Exp,
                                                 bias=nv[:, :], accum_out=s2[:, :])
                            nc.vector.tensor_tensor(s1[:, :], s1[:, :], s2[:, :],
                                                    ALU.add)
                            rc = TK.tile([128, 1], f32, name='rc', tag='rc')
                            nc.vector.reciprocal(rc[:, :], s1[:, :])
                            nc.vector.tensor_scalar(iww[:, :], iww[:, :], rc[:, :],
                                                    None, ALU.mult)
                        t1 = TK.tile([128, 64], f32, name='t1', tag='t1')
                        t2 = TK.tile([128, 64], f32, name='t2', tag='t2')
                        # lin = C + idx + 95*floor(idx/7); floor via >= ladder
                        nc.vector.tensor_scalar(t1[:, :], IW[:, 0:64], 6.5, 95.0,
                                                ALU.is_gt, ALU.mult)
                        for m in range(2, 7):
                            nc.vector.tensor_scalar(t2[:, :], IW[:, 0:64],
                                                    7.0 * m - 0.5, 95.0,
                                                    ALU.is_gt, ALU.mult)
                            nc.vector.tensor_tensor(t1[:, :], t1[:, :], t2[:, :],
                                                    ALU.add)
                        nc.vector.tensor_tensor(t1[:, :], t1[:, :], IW[:, 0:64],
                                                ALU.add)
                        nc.vector.tensor_scalar(IW[:, 0:64], t1[:, :],
                                                CS[:, ci:ci + 1], None, ALU.add)
                        nc.vector.tensor_scalar(IW[:, 64:128], IW[:, 64:128],
                                                PM[:, ci:ci + 1], None, ALU.mult)
                        TIW = TKP.tile([128, 128], f32, name='TIW', tag='TIW')
                        nc.tensor.transpose(TIW[:, :], IW[:, :], ID[:, :])
                        ITS = TK.tile([64, 128], f32, name='ITS', tag='ITS')
                        nc.vector.tensor_copy(ITS[:, :], TIW[0:64, :])
                        nc.vector.tensor_copy(WT[:, c0:c0 + 128], TIW[64:128, :])
                        PW = W16P.tile([128, 512], f32, name='PW', tag='PW')
                        for f in range(8):
                            IR = TK.tile([64, 128], f32, name='IR', tag='IR')
                            nc.vector.tensor_copy(IR[:, 0:16],
                                                  ITS[:, 16 * f:16 * f + 16])
                            nc.vector.tensor_copy(IR[:, 16:32], IR[:, 0:16])
                            nc.vector.tensor_copy(IR[:, 32:64], IR[:, 0:32])
                            nc.vector.tensor_copy(IR[:, 64:128], IR[:, 0:64])
                            nc.tensor.matmul(PW[:, 64 * f:64 * f + 64], IR[:, :],
                                             ID[0:64, 0:64], start=True, stop=True)
                        pwv = PW[:, :].rearrange('p (f c) -> p f c', c=64)
                        for h in range(4):
                            src = pwv[32 * h:32 * h + 32, :, :].rearrange(
                                'p f (k h2) -> p h2 k f', h2=4)[:, h, :, :]
                            nc.vector.tensor_copy(
                                w16v[32 * h:32 * h + 32, :, 8 * ci:8 * ci + 8], src)
                    if debug:
                        nc.sync.dma_start(out=ddbg['d_wt'][:, :], in_=WT[:, :])
                        nc.sync.dma_start(out=ddbg['d_w16'][:, :], in_=W16[:, :])
                        nc.sync.dma_start(out=ddbg['d_vp'][:, :], in_=VP[:, :])

            # ================= stage 4: gather + conv =================
            with (
                tc.tile_pool(name='g', bufs=1) as GP,
                tc.tile_pool(name='cv', bufs=2) as CV,
                tc.tile_pool(name='cvp', bufs=2, space='PSUM') as CVP,
            ):
                outv = dout[:, :].rearrange('p (y c) -> p y c', c=W)
                Lmax = max(bd[1] for bd in BANDS)
                for bi, (j0, L, f0, Fb, y0, nry) in reversed(list(enumerate(BANDS))):
                    Gs = []
                    for k in range(K):
                        G = GP.tile([128, Lmax], bf16, name=f'G{k}', tag=f'G{k}')
                        for c0 in range(0, L, 1024):
                            w = min(1024, L - c0)
                            GT = CV.tile([128, 1024], f32, name='GT', tag='GT')
                            nc.gpsimd.ap_gather(
                                GT[:, 0:w], VP[:, :],
                                w16v[:, k, f0 + c0 // 16:f0 + (c0 + w) // 16],
                                channels=128, num_elems=VPN, d=1, num_idxs=w)
                            for s0 in range(0, w, 512):
                                sw = min(512, w - s0)
                                PWB = CVP.tile([128, 512], f32, name='PWB',
                                               tag='PWB')
                                nc.tensor.matmul(
                                    PWB[:, 0:sw], SEL[:, 128 * k:128 * k + 128],
                                    WT[:, j0 + c0 + s0:j0 + c0 + s0 + sw],
                                    start=True, stop=True)
                                nc.vector.tensor_tensor(GT[:, s0:s0 + sw],
                                                        GT[:, s0:s0 + sw],
                                                        PWB[:, 0:sw], ALU.mult)
                            nc.vector.tensor_copy(G[:, c0:c0 + w], GT[:, 0:w])
                        Gs.append(G)
                    if debug and bi == 1:
                        nc.sync.dma_start(out=ddbg['d_g'][:, :],
                                          in_=Gs[0][:, 0:BANDS[1][1]])
                    yy = 0
                    while yy < nry:
                        cr = min(5, nry - yy)
                        F = cr * W
                        PO = CVP.tile([128, 5 * W], f32, name='PO', tag='PO')
                        n = 0
                        for k in range(K):
                            for d in range(9):
                                dy, dx = d // 3, d % 3
                                off = (y0 + yy + dy) * WP + dx - j0
                                rhs = Gs[k][:, off:off + cr * WP].rearrange(
                                    'p (r c) -> p r c', c=WP)[:, :, 0:W]
                                nc.tensor.matmul(PO[:, 0:F], clv[:, 9 * k + d, :],
                                                 rhs, start=(n == 0), stop=(n == 143))
                                n += 1
                        OSB = CV.tile([128, 5 * W], f32, name='OSB', tag='OSB')
                        nc.vector.tensor_scalar(OSB[:, 0:F], PO[:, 0:F], PB[:, :],
                                                None, ALU.add)
                        nc.sync.dma_start(
                            out=outv[:, y0 + yy:y0 + yy + cr, :],
                            in_=OSB[:, 0:F].rearrange('p (r c) -> p r c', c=W))
                        yy += cr
    nc.compile()
    return nc


# ======================= host side =======================

def host_inputs(vid, ln_w, ln_b, wq, bq, wk, bk, wv, bv, proj_w, proj_b):
    """Build the 8 per-core input dicts."""
    import ml_dtypes
    bf = ml_dtypes.bfloat16
    vid = np.asarray(vid, np.float32)

    def prep_w(w, b):
        wp = (np.asarray(w, np.float32) * np.asarray(ln_w, np.float32)[None, :])
        beta = np.asarray(w, np.float32) @ np.asarray(ln_b, np.float32) + \
            np.asarray(b, np.float32)
        return np.ascontiguousarray(wp.T), beta.reshape(1, 128)

    wqT, bqr = prep_w(wq, bq)
    wkT, bkr = prep_w(wk, bk)
    wvT, bvr = prep_w(wv, bv)

    pw = np.asarray(proj_w, np.float32)          # (128, 32, 16, 3, 3)
    cw = np.zeros((128, 144, 32), np.float32)
    for h in range(4):
        for i in range(HD):
            for k in range(K):
                for d in range(9):
                    cw[32 * h + i, 9 * k + d, :] = pw[32 * h:32 * h + 32, i, k,
                                                      d // 3, d % 3]
    cw = cw.reshape(128, 144 * 32).astype(bf)

    ident = np.eye(128, dtype=np.float32)
    cs = np.zeros((128, NTILE), np.float32)
    pms = [np.zeros((128, NTILE), np.float32) for _ in range(2)]
    for ci in range(NTILE):
        for p in range(128):
            j = 128 * ci + p
            s, xp = j // WP, j % WP
            if j < NPX and 1 <= xp <= 96:
                cs[p, ci] = (s + 1) * VPC + (xp - 1) + 0.25
                for half in range(2):
                    if 0 <= 48 * half + s - 1 < H:
                        pms[half][p, ci] = 1.0
    eba = np.array([[-EPS_TIE * (p // 4)] for p in range(128)], np.float32)
    selsrc = np.zeros((4, 128), np.float32)
    for h in range(4):
        selsrc[h, 32 * h:32 * h + 32] = 1.0
    selsrc = selsrc.astype(bf)
    ebb = np.array([[-EPS_TIE * (32 + p // 4)] for p in range(68)], np.float32)
    pbr = np.asarray(proj_b, np.float32).reshape(128, 1)

    maps = []
    for core in range(8):
        t, half = core // 2, core % 2
        y0 = 48 * half
        slab = np.zeros((SLAB, 128, W), np.float32)
        valid = np.zeros((SLAB, W), np.float32)
        for r in range(SLAB):
            g = y0 - 5 + r
            if 0 <= g < H:
                slab[r] = vid[0, t, :, g, :]
                valid[r] = 1.0
        maps.append({
            'vid': np.ascontiguousarray(slab.transpose(1, 0, 2)).reshape(
                128, SLAB * W),
            'wqT': wqT, 'wkT': wkT, 'wvT': wvT,
            'bq': bqr, 'bk': bkr, 'bv': bvr,
            'valid': valid.reshape(1, SLAB * W),
            'cw': cw, 'ident': ident, 'cs': cs, 'pm': pms[half],
            'selsrc': selsrc,
            'eba': eba, 'ebb': ebb, 'projb': pbr,
        })
    return maps


_CACHE = {}


def kernel(vid, ln_w, ln_b, wq, bq, wk, bk, wv, bv, proj_w, proj_b):
    from concourse.bass_utils import run_bass_kernel_spmd
    maps = host_inputs(vid, ln_w, ln_b, wq, bq, wk, bk, wv, bv, proj_w, proj_b)
    if 'nc' not in _CACHE:
        _CACHE['nc'] = build_nc()
    res = run_bass_kernel_spmd(_CACHE['nc'], maps, list(range(8)))
    out = np.zeros((T, C, H, W), np.float32)
    for core in range(8):
        t, half = core // 2, core % 2
        out[t, :, 48 * half:48 * half + 48, :] = \
            res.results[core]['out'].reshape(128, 48, W)
    return out.reshape(B, T, C, H, W)


# revision 5
# speedup vs baseline: 5.5571x; 1.2111x over previous
"""Full on-device NonLocalAttentionStack kernel for 8 trn2 cores.

Sharding: 8 cores = 4 frames x 2 row-halves (48 out rows each). Per core:
LN -> QKV (PE) -> 49-offset patch search (vector mult + PE ones-reduce +
box sums) -> top-16 via max/max_index/match_replace (pixel-major after PE
transposes) -> softmax (scalar Exp) -> wrapped index build (replicating
transpose-matmuls) -> GPSIMD indirect_copy gather of v (bf16) -> weight
multiply -> grouped Conv3d as 144 PSUM-accumulated block-diag matmuls.
"""
import numpy as np

NH, WS, PS, K = 4, 7, 3, 16
B, T, C, H, W = 1, 4, 128, 96, 96
HD = C // NH                    # 32
SLAB = 58                       # slab rows [-5, 53) around the 48-row half
NS = 50                         # score/stack rows s=0..49 (spatial y = s-1)
WP = 98                         # padded x grid (x = -1..96)
NPX = NS * WP                   # 4900
NTILE = 39
NPXP = NTILE * 128              # 4992
VPC = 102                       # vpad cols (x = -3..98)
VPN = SLAB * VPC                # 5916
EPS_TIE = 1e-5
OUTR = 48
NF = NPXP // 16                 # 312


def _bands():
    b = []
    for y0, y1 in ((0, 24), (24, 48)):
        j_lo = (y0 * WP) // 1024 * 1024
        j_hi = -((-((y1 + 2) * WP)) // 16) * 16
        b.append((j_lo, j_hi - j_lo, j_lo // 16, (j_hi - j_lo) // 16, y0, y1 - y0))
    return b


BANDS = _bands()


def build_nc(debug=False):
    import concourse.bacc as bacc
    import concourse.mybir as mybir
    from concourse.tile import TileContext
    f32, bf16, u16, u32 = (mybir.dt.float32, mybir.dt.bfloat16,
                           mybir.dt.int16, mybir.dt.uint32)
    f16 = mybir.dt.float16
    AF, ALU = mybir.ActivationFunctionType, mybir.AluOpType

    nc = bacc.Bacc()
    din = {}
    for nm, shp, dt in [
        ('vid', [128, SLAB * W], f16), ('wqT', [128, 128], f32),
        ('wkT', [128, 128], f32), ('wvT', [128, 128], f32),
        ('bq', [1, 128], f32), ('bk', [1, 128], f32), ('bv', [1, 128], f32),
        ('valid', [1, SLAB * W], f32), ('cw', [128, 144 * 32], bf16),
        ('ident', [128, 128], f32), ('cs', [128, NTILE], f32),
        ('pm', [128, NTILE], f32), ('eba', [128, 1], f32),
        ('selsrc', [4, 128], bf16),
        ('ebb', [68, 1], f32), ('projb', [128, 1], f32),
    ]:
        din[nm] = nc.declare_dram_parameter(nm, shp, dt, isOutput=False)
    dout = nc.declare_dram_parameter('out', [128, OUTR * W], f16, isOutput=True)
    ddbg = {}
    if debug:
        for nm, shp, dt in [('d_sca', [128, NPXP], f32), ('d_scb', [68, NPXP], f32),
                            ('d_wt', [64, NPXP], bf16), ('d_w16', [128, K * NF], u16),
                            ('d_q', [128, SLAB * W], f32), ('d_vp', [128, VPN], f32),
                            ('d_g', [128, BANDS[1][1]], bf16)]:
            ddbg[nm] = nc.declare_dram_parameter(nm, shp, dt, isOutput=True)

    with TileContext(nc) as tc:
        with (
            tc.tile_pool(name='persist', bufs=1) as PP,
            tc.tile_pool(name='const', bufs=1) as CP,
        ):
            # ---- constants ----
            ID = CP.tile([128, 128], f32, name='ID', tag='ID')
            nc.sync.dma_start(out=ID[:, :], in_=din['ident'][:, :])
            CS = CP.tile([128, NTILE], f32, name='CS', tag='CS')
            nc.sync.dma_start(out=CS[:, :], in_=din['cs'][:, :])
            PM = CP.tile([128, NTILE], f32, name='PM', tag='PM')
            nc.sync.dma_start(out=PM[:, :], in_=din['pm'][:, :])
            EBA = CP.tile([128, 1], f32, name='EBA', tag='EBA')
            nc.sync.dma_start(out=EBA[:, :], in_=din['eba'][:, :])
            EBB = CP.tile([68, 1], f32, name='EBB', tag='EBB')
            nc.sync.dma_start(out=EBB[:, :], in_=din['ebb'][:, :])
            PB = CP.tile([128, 1], f32, name='PB', tag='PB')
            nc.sync.dma_start(out=PB[:, :], in_=din['projb'][:, :])
            WQT = CP.tile([128, 128], f32, name='WQT', tag='WQT')
            nc.sync.dma_start(out=WQT[:, :], in_=din['wqT'][:, :])
            WKT = CP.tile([128, 128], f32, name='WKT', tag='WKT')
            nc.sync.dma_start(out=WKT[:, :], in_=din['wkT'][:, :])
            WVT = CP.tile([128, 128], f32, name='WVT', tag='WVT')
            nc.sync.dma_start(out=WVT[:, :], in_=din['wvT'][:, :])
            BQ = CP.tile([1, 128], f32, name='BQ', tag='BQ')
            nc.sync.dma_start(out=BQ[:, :], in_=din['bq'][:, :])
            BK = CP.tile([1, 128], f32, name='BK', tag='BK')
            nc.sync.dma_start(out=BK[:, :], in_=din['bk'][:, :])
            BV = CP.tile([1, 128], f32, name='BV', tag='BV')
            nc.sync.dma_start(out=BV[:, :], in_=din['bv'][:, :])
            EPSC = CP.tile([1, 1], f32, name='EPSC', tag='EPSC')
            nc.vector.memset(EPSC[:, :], 1e-6)
            ONES1 = CP.tile([1, 128], f32, name='ONES1', tag='ONES1')
            nc.vector.memset(ONES1[:, :], 1.0)
            ONE128 = CP.tile([128, 1], f32, name='ONE128', tag='ONE128')
            nc.vector.memset(ONE128[:, :], 1.0)
            ZB = CP.tile([128, 252], f32, name='ZB', tag='ZB')
            nc.vector.memset(ZB[:, :], 0.0)
            for h in range(4):
                nc.vector.memset(ZB[32 * h:32 * h + 32, 124 + h:125 + h], 1.0)
            SEL = CP.tile([64, K * 128], bf16, name='SEL', tag='SEL')
            nc.vector.memset(SEL[:, :], 0.0)
            for k in range(K):
                nc.sync.dma_start(out=SEL[4 * k:4 * k + 4, 128 * k:128 * k + 128],
                                  in_=din['selsrc'][:, :])

            # ---- persistent data ----
            CL = PP.tile([128, 144 * 128], bf16, name='CL', tag='CL')
            VP = PP.tile([128, VPN], f32, name='VP', tag='VP')
            W16 = PP.tile([128, K * NF], u16, name='W16', tag='W16')
            WT = PP.tile([64, NPXP], bf16, name='WT', tag='WT')
            nc.vector.memset(VP[:, :], 0.0)
            clv = CL[:, :].rearrange('p (m c) -> p m c', c=128)
            vview = VP[:, :].rearrange('p (r c) -> p r c', c=VPC)
            w16v = W16[:, :].rearrange('p (k f) -> p k f', f=NF)

            with tc.tile_pool(name='cwp', bufs=1) as CWP:
                CW = CWP.tile([128, 144 * 32], bf16, name='CW', tag='CW')
                nc.sync.dma_start(out=CW[:, :], in_=din['cw'][:, :])
                nc.vector.memset(CL[:, :], 0.0)
                cwv = CW[:, :].rearrange('p (m c) -> p m c', c=32)
                for m in range(144):
                    for h in range(4):
                        nc.vector.tensor_copy(
                            clv[32 * h:32 * h + 32, m, 32 * h:32 * h + 32],
                            cwv[32 * h:32 * h + 32, m, :])

            # ================= stages 1-3 =================
            with tc.tile_pool(name='sc', bufs=1) as SC:
                SCA = SC.tile([128, NPXP], f32, name='SCA', tag='SCA')
                SCB = SC.tile([68, NPXP], f32, name='SCB', tag='SCB')
                nc.vector.memset(SCA[:, :], 0.0)
                nc.vector.memset(SCB[:, :], 0.0)
                scav = SCA[:, 0:NPX].rearrange('p (s c) -> p s c', c=WP)
                scbv = SCB[:, 0:NPX].rearrange('p (s c) -> p s c', c=WP)

                with tc.tile_pool(name='qk', bufs=1) as QK:
                    Q = QK.tile([128, SLAB * W], f32, name='Q', tag='Q')
                    KP = QK.tile([128, SLAB * VPC], f32, name='KP', tag='KP')
                    nc.vector.memset(KP[:, :], 0.0)
                    kview = KP[:, :].rearrange('p (r c) -> p r c', c=VPC)
                    qview = Q[:, :].rearrange('p (r c) -> p r c', c=W)

                    # ----- stage 1: LN + QKV -----
                    with (
                        tc.tile_pool(name='ln', bufs=2) as LN,
                        tc.tile_pool(name='lnp', bufs=1, space='PSUM') as LNP,
                        tc.tile_pool(name='qkvp', bufs=1, space='PSUM') as QKVP,
                    ):
                        r0 = 0
                        while r0 < SLAB:
                            nr = min(4, SLAB - r0)
                            F = nr * W
                            xch = LN.tile([128, 4 * W], f16, name='xch',
                                          tag='xch')
                            nc.sync.dma_start(
                                out=xch[:, 0:F],
                                in_=din['vid'][:, r0 * W:r0 * W + F])
                            xc = LN.tile([128, 4 * W], f32, name='xc', tag='xc')
                            nc.vector.tensor_copy(xc[:, 0:F], xch[:, 0:F])
                            vlc = LN.tile([1, 4 * W], f32, name='vlc', tag='vlc')
                            nc.sync.dma_start(
                                out=vlc[:, 0:F],
                                in_=din['valid'][:, r0 * W:r0 * W + F])
                            sq = LN.tile([128, 4 * W], f32, name='sq', tag='sq')
                            nc.scalar.square(sq[:, 0:F], xc[:, 0:F])
                            ps1 = LNP.tile([1, 4 * W], f32, name='ps1', tag='ps1')
                            ps2 = LNP.tile([1, 4 * W], f32, name='ps2', tag='ps2')
                            nc.tensor.matmul(ps1[:, 0:F], ONE128[:, :], xc[:, 0:F],
                                             start=True, stop=True)
                            nc.tensor.matmul(ps2[:, 0:F], ONE128[:, :], sq[:, 0:F],
                                             start=True, stop=True)
                            mu = LN.tile([1, 4 * W], f32, name='mu', tag='mu', bufs=1)
                            nc.vector.tensor_scalar(mu[:, 0:F], ps1[:, 0:F],
                                                    1.0 / 128, None, ALU.mult)
                            var = LN.tile([1, 4 * W], f32, name='var', tag='var', bufs=1)
                            nc.vector.tensor_scalar(var[:, 0:F], ps2[:, 0:F],
                                                    1.0 / 128, None, ALU.mult)
                            mu2 = LN.tile([1, 4 * W], f32, name='mu2', tag='mu2', bufs=1)
                            nc.scalar.square(mu2[:, 0:F], mu[:, 0:F])
                            nc.vector.tensor_tensor(var[:, 0:F], var[:, 0:F],
                                                    mu2[:, 0:F], ALU.subtract)
                            sd = LN.tile([1, 4 * W], f32, name='sd', tag='sd', bufs=1)
                            nc.scalar.activation(sd[:, 0:F], var[:, 0:F], AF.Sqrt,
                                                 bias=EPSC[:, :])
                            rs = LN.tile([1, 4 * W], f32, name='rs', tag='rs', bufs=1)
                            nc.vector.reciprocal(rs[:, 0:F], sd[:, 0:F])
                            pmu = LNP.tile([128, 4 * W], f32, name='pmu', tag='pmu')
                            prs = LNP.tile([128, 4 * W], f32, name='prs', tag='prs')
                            nc.tensor.matmul(pmu[:, 0:F], ONES1[:, :], mu[:, 0:F],
                                             start=True, stop=True)
                            nc.tensor.matmul(prs[:, 0:F], ONES1[:, :], rs[:, 0:F],
                                             start=True, stop=True)
                            xn = LN.tile([128, 4 * W], f32, name='xn', tag='xn')
                            nc.vector.tensor_tensor(xn[:, 0:F], xc[:, 0:F],
                                                    pmu[:, 0:F], ALU.subtract)
                            nc.vector.tensor_tensor(xn[:, 0:F], xn[:, 0:F],
                                                    prs[:, 0:F], ALU.mult)
                            for wt_ap, b_ap, dst in ((WQT, BQ, 'q'), (WKT, BK, 'k'),
                                                     (WVT, BV, 'v')):
                                pq = QKVP.tile([128, 4 * W], f32, name=f'p{dst}',
                                               tag=f'p{dst}')
                                nc.tensor.matmul(pq[:, 0:F], wt_ap[:, :], xn[:, 0:F],
                                                 start=True, stop=False)
                                nc.tensor.matmul(pq[:, 0:F], b_ap[:, :],
                                                 vlc[:, 0:F],
                                                 start=False, stop=True)
                                pqv = pq[:, 0:F].rearrange('p (r c) -> p r c', c=W)
                                if dst == 'q':
                                    nc.vector.tensor_copy(qview[:, r0:r0 + nr, :],
                                                          pqv)
                                elif dst == 'k':
                                    nc.vector.tensor_copy(
                                        kview[:, r0:r0 + nr, 3:99], pqv)
                                else:
                                    nc.vector.tensor_copy(
                                        vview[:, r0:r0 + nr, 3:99], pqv)
                            r0 += nr

                    # ----- stage 2: search + scores -----
                    with (
                        tc.tile_pool(name='pr', bufs=3) as PR,
                        tc.tile_pool(name='ipp', bufs=1, space='PSUM') as IPP,
                        tc.tile_pool(name='xbp', bufs=2) as XBP,
                    ):
                        for s0 in range(0, NS, 3):
                            nr = min(3, NS - s0)
                            ipr = nr + 2
                            F = ipr * W
                            ipA = IPP.tile([128, 5 * W], f32, name='ipA', tag='ipA')
                            ipB = IPP.tile([68, 5 * W], f32, name='ipB', tag='ipB')
                            for o in range(49):
                                dy, dx = o // 7, o % 7
                                P = PR.tile([128, 5 * W], f32, name='P', tag='P')
                                nc.vector.tensor_tensor(
                                    P[:, 0:F].rearrange('p (r c) -> p r c', c=W),
                                    qview[:, s0 + 3:s0 + 3 + ipr, :],
                                    kview[:, s0 + dy:s0 + dy + ipr, dx:dx + W],
                                    ALU.mult)
                                if o < 32:
                                    nc.tensor.matmul(
                                        ipA[:, 0:F], ZB[:, 124 - 4 * o:252 - 4 * o],
                                        P[:, 0:F], start=(o == 0), stop=(o == 31))
                                else:
                                    o2 = o - 32
                                    nc.tensor.matmul(
                                        ipB[:, 0:F],
                                        ZB[:, 124 - 4 * o2:192 - 4 * o2],
                                        P[:, 0:F], start=(o == 32), stop=(o == 48))
                            for (ip, scv, nprt, eb) in ((ipA, scav, 128, EBA),
                                                        (ipB, scbv, 68, EBB)):
                                ips = XBP.tile([128, 5 * W], f32, name='ips',
                                               tag='ips')
                                nc.vector.tensor_copy(ips[0:nprt, 0:F],
                                                      ip[0:nprt, 0:F])
                                ipv = ips[0:nprt, 0:F].rearrange('p (r c) -> p r c',
                                                                 c=W)
                                xb = XBP.tile([128, 5 * WP], f32, name='xb', tag='xb')
                                xbv = xb[0:nprt, 0:ipr * WP].rearrange(
                                    'p (r c) -> p r c', c=WP)
                                nc.vector.tensor_tensor(xbv[:, :, 2:96],
                                                        ipv[:, :, 0:94],
                                                        ipv[:, :, 1:95], ALU.add)
                                nc.vector.tensor_tensor(xbv[:, :, 2:96],
                                                        xbv[:, :, 2:96],
                                                        ipv[:, :, 2:96], ALU.add)
                                nc.vector.tensor_tensor(xbv[:, :, 1:2],
                                                        ipv[:, :, 0:1],
                                                        ipv[:, :, 1:2], ALU.add)
                                nc.vector.tensor_tensor(xbv[:, :, 96:97],
                                                        ipv[:, :, 94:95],
                                                        ipv[:, :, 95:96], ALU.add)
                                nc.vector.tensor_tensor(scv[0:nprt, s0:s0 + nr, 1:97],
                                                        xbv[:, 0:nr, 1:97],
                                                        xbv[:, 1:nr + 1, 1:97],
                                                        ALU.add)
                                nc.vector.tensor_tensor(scv[0:nprt, s0:s0 + nr, 1:97],
                                                        scv[0:nprt, s0:s0 + nr, 1:97],
                                                        xbv[:, 2:nr + 2, 1:97],
                                                        ALU.add)
                                nc.vector.tensor_scalar(scv[0:nprt, s0:s0 + nr, 1:97],
                                                        scv[0:nprt, s0:s0 + nr, 1:97],
                                                        eb[0:nprt, :], None, ALU.add)
                    if debug:
                        nc.sync.dma_start(out=ddbg['d_sca'][:, :], in_=SCA[:, :])
                        nc.sync.dma_start(out=ddbg['d_scb'][:, :], in_=SCB[:, :])
                        nc.sync.dma_start(out=ddbg['d_q'][:, :], in_=Q[:, :])

                # ----- stage 3: transpose + topk + softmax + wrapped idx -----
                with (
                    tc.tile_pool(name='tk', bufs=2) as TK,
                    tc.tile_pool(name='tkp', bufs=1, space='PSUM') as TKP,
                    tc.tile_pool(name='w16p', bufs=2, space='PSUM') as W16P,
                ):
                    for ci in range(NTILE):
                        c0 = 128 * ci
                        T1 = TKP.tile([128, 128], f32, name='T1', tag='T1')
                        nc.tensor.transpose(T1[:, :], SCA[:, c0:c0 + 128], ID[:, :])
                        T2 = TKP.tile([128, 68], f32, name='T2', tag='T2')
                        nc.tensor.transpose(T2[:, :], SCB[:, c0:c0 + 128],
                                            ID[0:68, 0:68])
                        S = TK.tile([128, 196], f32, name='S', tag='S')
                        nc.vector.tensor_copy(S[:, 0:128], T1[:, :])
                        nc.vector.tensor_copy(S[:, 128:196], T2[:, :])
                        IW = TK.tile([128, 128], f32, name='IW', tag='IW')
                        for h in range(4):
                            hv = S[:, :].rearrange('p (o h) -> p h o', h=4)[:, h, :]
                            m1 = TK.tile([128, 8], f32, name='m1', tag='m1')
                            m2 = TK.tile([128, 8], f32, name='m2', tag='m2')
                            i1 = TK.tile([128, 8], u32, name='i1', tag='i1')
                            i2 = TK.tile([128, 8], u32, name='i2', tag='i2')
                            nc.vector.max(m1[:, :], hv)
                            nc.vector.max_index(i1[:, :], m1[:, :], hv)
                            nc.vector.match_replace(hv, m1[:, :], hv, -1e30)
                            nc.vector.max(m2[:, :], hv)
                            nc.vector.max_index(i2[:, :], m2[:, :], hv)
                            iwi = IW[:, 0:64].rearrange('p (k h) -> p h k',
                                                        h=4)[:, h, :]
                            nc.vector.tensor_copy(iwi[:, 0:8], i1[:, :])
                            nc.vector.tensor_copy(iwi[:, 8:16], i2[:, :])
                            nv = TK.tile([128, 1], f32, name='nv', tag='nv')
                            nc.vector.tensor_scalar(nv[:, :], m1[:, 0:1], -1.0,
                                                    None, ALU.mult)
                            iww = IW[:, 64:128].rearrange('p (k h) -> p h k',
                                                          h=4)[:, h, :]
                            s1 = TK.tile([128, 1], f32, name='s1', tag='s1')
                            s2 = TK.tile([128, 1], f32, name='s2', tag='s2')
                            nc.scalar.activation(iww[:, 0:8], m1[:, :], AF.Exp,
                                                 bias=nv[:, :], accum_out=s1[:, :])
                            nc.scalar.activation(iww[:, 8:16], m2[:, :], AF.Exp,
                                                 bias=nv[:, :], accum_out=s2[:, :])
                            nc.vector.tensor_tensor(s1[:, :], s1[:, :], s2[:, :],
                                                    ALU.add)
                            rc = TK.tile([128, 1], f32, name='rc', tag='rc')
                            nc.vector.reciprocal(rc[:, :], s1[:, :])
                            nc.vector.tensor_scalar(iww[:, :], iww[:, :], rc[:, :],
                                                    None, ALU.mult)
                        t1 = TK.tile([128, 64], f32, name='t1', tag='t1')
                        t2 = TK.tile([128, 64], f32, name='t2', tag='t2')
                        # lin = C + idx + 95*floor(idx/7); floor via >= ladder
                        nc.vector.tensor_scalar(t1[:, :], IW[:, 0:64], 6.5, 95.0,
                                                ALU.is_gt, ALU.mult)
                        for m in range(2, 7):
                            nc.vector.tensor_scalar(t2[:, :], IW[:, 0:64],
                                                    7.0 * m - 0.5, 95.0,
                                                    ALU.is_gt, ALU.mult)
                            nc.vector.tensor_tensor(t1[:, :], t1[:, :], t2[:, :],
                                                    ALU.add)
                        nc.vector.tensor_tensor(t1[:, :], t1[:, :], IW[:, 0:64],
                                                ALU.add)
                        nc.vector.tensor_scalar(IW[:, 0:64], t1[:, :],
                                                CS[:, ci:ci + 1], None, ALU.add)
                        nc.vector.tensor_scalar(IW[:, 64:128], IW[:, 64:128],
                                                PM[:, ci:ci + 1], None, ALU.mult)
                        TIW = TKP.tile([128, 128], f32, name='TIW', tag='TIW')
                        nc.tensor.transpose(TIW[:, :], IW[:, :], ID[:, :])
                        ITS = TK.tile([64, 128], f32, name='ITS', tag='ITS')
                        nc.vector.tensor_copy(ITS[:, :], TIW[0:64, :])
                        nc.vector.tensor_copy(WT[:, c0:c0 + 128], TIW[64:128, :])
                        PW = W16P.tile([128, 512], f32, name='PW', tag='PW')
                        for f in range(8):
                            IR = TK.tile([64, 128], f32, name='IR', tag='IR')
                            nc.vector.tensor_copy(IR[:, 0:16],
                                                  ITS[:, 16 * f:16 * f + 16])
                            nc.vector.tensor_copy(IR[:, 16:32], IR[:, 0:16])
                            nc.vector.tensor_copy(IR[:, 32:64], IR[:, 0:32])
                            nc.vector.tensor_copy(IR[:, 64:128], IR[:, 0:64])
                            nc.tensor.matmul(PW[:, 64 * f:64 * f + 64], IR[:, :],
                                             ID[0:64, 0:64], start=True, stop=True)
                        pwv = PW[:, :].rearrange('p (f c) -> p f c', c=64)
                        for h in range(4):
                            src = pwv[32 * h:32 * h + 32, :, :].rearrange(
                                'p f (k h2) -> p h2 k f', h2=4)[:, h, :, :]
                            nc.vector.tensor_copy(
                                w16v[32 * h:32 * h + 32, :, 8 * ci:8 * ci + 8], src)
                    if debug:
                        nc.sync.dma_start(out=ddbg['d_wt'][:, :], in_=WT[:, :])
                        nc.sync.dma_start(out=ddbg['d_w16'][:, :], in_=W16[:, :])
                        nc.sync.dma_start(out=ddbg['d_vp'][:, :], in_=VP[:, :])

            # ================= stage 4: gather + conv =================
            with (
                tc.tile_pool(name='g', bufs=1) as GP,
                tc.tile_pool(name='cv', bufs=2) as CV,
                tc.tile_pool(name='cvp', bufs=2, space='PSUM') as CVP,
            ):
                outv = dout[:, :].rearrange('p (y c) -> p y c', c=W)
                Lmax = max(bd[1] for bd in BANDS)
                for bi, (j0, L, f0, Fb, y0, nry) in reversed(list(enumerate(BANDS))):
                    Gs = []
                    for k in range(K):
                        G = GP.tile([128, Lmax], bf16, name=f'G{k}', tag=f'G{k}')
                        for c0 in range(0, L, 1024):
                            w = min(1024, L - c0)
                            GT = CV.tile([128, 1024], f32, name='GT', tag='GT')
                            nc.gpsimd.ap_gather(
                                GT[:, 0:w], VP[:, :],
                                w16v[:, k, f0 + c0 // 16:f0 + (c0 + w) // 16],
                                channels=128, num_elems=VPN, d=1, num_idxs=w)
                            for s0 in range(0, w, 512):
                                sw = min(512, w - s0)
                                PWB = CVP.tile([128, 512], f32, name='PWB',
                                               tag='PWB')
                                nc.tensor.matmul(
                                    PWB[:, 0:sw], SEL[:, 128 * k:128 * k + 128],
                                    WT[:, j0 + c0 + s0:j0 + c0 + s0 + sw],
                                    start=True, stop=True)
                                nc.vector.tensor_tensor(GT[:, s0:s0 + sw],
                                                        GT[:, s0:s0 + sw],
                                                        PWB[:, 0:sw], ALU.mult)
                            nc.vector.tensor_copy(G[:, c0:c0 + w], GT[:, 0:w])
                        Gs.append(G)
                    if debug and bi == 1:
                        nc.sync.dma_start(out=ddbg['d_g'][:, :],
                                          in_=Gs[0][:, 0:BANDS[1][1]])
                    yy = 0
                    while yy < nry:
                        cr = min(5, nry - yy)
                        F = cr * W
                        PO = CVP.tile([128, 5 * W], f32, name='PO', tag='PO')
                        n = 0
                        for k in range(K):
                            for d in range(9):
                                dy, dx = d // 3, d % 3
                                off = (y0 + yy + dy) * WP + dx - j0
                                rhs = Gs[k][:, off:off + cr * WP].rearrange(
                                    'p (r c) -> p r c', c=WP)[:, :, 0:W]
                                nc.tensor.matmul(PO[:, 0:F], clv[:, 9 * k + d, :],
                                                 rhs, start=(n == 0), stop=(n == 143))
                                n += 1
                        OSB = CV.tile([128, 5 * W], f16, name='OSB', tag='OSB')
                        nc.vector.tensor_scalar(OSB[:, 0:F], PO[:, 0:F], PB[:, :],
                                                None, ALU.add)
                        nc.sync.dma_start(
                            out=outv[:, y0 + yy:y0 + yy + cr, :],
                            in_=OSB[:, 0:F].rearrange('p (r c) -> p r c', c=W))
                        yy += cr
    nc.compile()
    return nc


# ======================= host side =======================

def host_inputs(vid, ln_w, ln_b, wq, bq, wk, bk, wv, bv, proj_w, proj_b):
    """Build the 8 per-core input dicts."""
    import ml_dtypes
    bf = ml_dtypes.bfloat16
    vid = np.asarray(vid, np.float32)

    def prep_w(w, b):
        wp = (np.asarray(w, np.float32) * np.asarray(ln_w, np.float32)[None, :])
        beta = np.asarray(w, np.float32) @ np.asarray(ln_b, np.float32) + \
            np.asarray(b, np.float32)
        return np.ascontiguousarray(wp.T), beta.reshape(1, 128)

    wqT, bqr = prep_w(wq, bq)
    wkT, bkr = prep_w(wk, bk)
    wvT, bvr = prep_w(wv, bv)

    pw = np.asarray(proj_w, np.float32)          # (128, 32, 16, 3, 3)
    cw = np.zeros((128, 144, 32), np.float32)
    for h in range(4):
        for i in range(HD):
            for k in range(K):
                for d in range(9):
                    cw[32 * h + i, 9 * k + d, :] = pw[32 * h:32 * h + 32, i, k,
                                                      d // 3, d % 3]
    cw = cw.reshape(128, 144 * 32).astype(bf)

    ident = np.eye(128, dtype=np.float32)
    cs = np.zeros((128, NTILE), np.float32)
    pms = [np.zeros((128, NTILE), np.float32) for _ in range(2)]
    for ci in range(NTILE):
        for p in range(128):
            j = 128 * ci + p
            s, xp = j // WP, j % WP
            if j < NPX and 1 <= xp <= 96:
                cs[p, ci] = (s + 1) * VPC + (xp - 1) + 0.25
                for half in range(2):
                    if 0 <= 48 * half + s - 1 < H:
                        pms[half][p, ci] = 1.0
    eba = np.array([[-EPS_TIE * (p // 4)] for p in range(128)], np.float32)
    selsrc = np.zeros((4, 128), np.float32)
    for h in range(4):
        selsrc[h, 32 * h:32 * h + 32] = 1.0
    selsrc = selsrc.astype(bf)
    ebb = np.array([[-EPS_TIE * (32 + p // 4)] for p in range(68)], np.float32)
    pbr = np.asarray(proj_b, np.float32).reshape(128, 1)

    maps = []
    for core in range(8):
        t, half = core // 2, core % 2
        y0 = 48 * half
        slab = np.zeros((SLAB, 128, W), np.float32)
        valid = np.zeros((SLAB, W), np.float32)
        for r in range(SLAB):
            g = y0 - 5 + r
            if 0 <= g < H:
                slab[r] = vid[0, t, :, g, :]
                valid[r] = 1.0
        maps.append({
            'vid': np.ascontiguousarray(slab.transpose(1, 0, 2)).reshape(
                128, SLAB * W).astype(np.float16),
            'wqT': wqT, 'wkT': wkT, 'wvT': wvT,
            'bq': bqr, 'bk': bkr, 'bv': bvr,
            'valid': valid.reshape(1, SLAB * W),
            'cw': cw, 'ident': ident, 'cs': cs, 'pm': pms[half],
            'selsrc': selsrc,
            'eba': eba, 'ebb': ebb, 'projb': pbr,
        })
    return maps


_CACHE = {}


def kernel(vid, ln_w, ln_b, wq, bq, wk, bk, wv, bv, proj_w, proj_b):
    from concourse.bass_utils import run_bass_kernel_spmd
    maps = host_inputs(vid, ln_w, ln_b, wq, bq, wk, bk, wv, bv, proj_w, proj_b)
    if 'nc' not in _CACHE:
        _CACHE['nc'] = build_nc()
    res = run_bass_kernel_spmd(_CACHE['nc'], maps, list(range(8)))
    out = np.zeros((T, C, H, W), np.float32)
    for core in range(8):
        t, half = core // 2, core % 2
        out[t, :, 48 * half:48 * half + 48, :] = \
            res.results[core]['out'].reshape(128, 48, W).astype(np.float32)
    return out.reshape(B, T, C, H, W)
